# revision 1
# baseline (speedup 1.0000x reference)
"""Trainium2 Bass kernel for nn_BDH_6313601925221 (sparse_attention).

Model (reference.py):
  x = LN(embed[idx])                                   (B=1, T=1024, D=256)
  repeat 6 layers (shared weights):
    x_sparse = relu(einsum('btd,hdn->bhtn', x, encoder))   N=8192, NH=4
    QR       = rope(x_sparse)                              interleaved-pair rotation
    scores   = einsum('bhtn,bhsn->bhts', QR, QR) * strict_causal
    yKV      = LN(einsum('bhts,bsd->bhtd', scores, x))
    y_sparse = relu(einsum('bhtd,hdn->bhtn', yKV, encoder_v))
    yMLP     = (x_sparse*y_sparse).transpose -> (T, NH*N) @ decoder
    x        = LN(x + LN(yMLP))
  logits = x @ lm_head

Distribution (8 cores): core c = (head h=c//2, latent-half eta=c%2).
Each core computes the encoder/rope/scores path over its 4096 latent dims
(pairwise AllReduce of partial scores within the head pair), duplicates the
small yKV path, then computes y_sparse/xy/decoder over its latent half for
all tokens; one 8-rank AllReduce of the yMLP partials per layer.

Layouts: latent dim N is host-permuted so rope pairs are de-interleaved:
local tile 2j = even pair members, 2j+1 = odd. Inner products over N and
the decoder contraction are invariant to this permutation (weights are
permuted to match).

PSUM budget (8 banks): acc_a/acc_b/acc_c [128,1024] f32 (2 banks each,
bufs=1) carry all long-lived accumulations (score strips, yKV, yMLP);
ps_w [128,512] (bufs=2) carries transient matmul outputs.
"""

import math
import sys

import numpy as np

for _p in ("/opt/trn_rl_repo",):
    if _p not in sys.path:
        sys.path.insert(0, _p)

import concourse.bass as bass
import concourse.mybir as mybir
import concourse.tile as tile
from concourse import bacc
from concourse import bass_utils

# ---------------------------------------------------------------- constants
D = 256
NH = 4
N = 8192
T = 1024
N_LAYER = 6
VOCAB = 256
THETA = 2 ** 16
EPS = 1e-5
NCORES = 8

NHALF = N // 2          # 4096 latent dims per core
NPAIR = NHALF // 2      # 2048 rope pairs per core
NT = NHALF // 128       # 32 local n-tiles of 128
NJ = NT // 2            # 16 pair-blocks (tile 2j = evens, 2j+1 = odds)
TB = T // 128           # 8 token blocks
DC = D // 128           # 2 d-chunks

F16 = mybir.dt.float16
F32 = mybir.dt.float32
I32 = mybir.dt.int32
AX = mybir.AxisListType
ALU = mybir.AluOpType
ACTF = mybir.ActivationFunctionType

# kb -> (group, acc tag, column offset inside the [128,1024] acc tile)
SC_LAYOUT = {
    0: (0, "acc_a", 0),
    1: (0, "acc_b", 0),
    2: (0, "acc_c", 0),
    3: (1, "acc_a", 0),
    4: (1, "acc_b", 0),
    5: (1, "acc_b", 512),
    6: (1, "acc_c", 0),
    7: (1, "acc_c", 512),
}


def _bi(kb, qb):
    """Linear index of score block (kb, qb), kb <= qb."""
    return kb * TB - (kb * (kb - 1)) // 2 + (qb - kb)


def _ln_free(nc, pool, x_ap, eps_ap, out_f32=None, out_f16=None,
             skip_mean=False, n=None, name=""):
    """LayerNorm along the free dim of a [128, n] tile (per-partition stats)."""
    n = n if n is not None else x_ap.shape[-1]
    inv_n = 1.0 / n
    sq = pool.tile([128, n], F32, name=f"lnsq{name}", tag="lnsq")
    ssq = pool.tile([128, 1], F32, name=f"lnssq{name}", tag="lnssq")
    std = pool.tile([128, 1], F32, name=f"lnstd{name}", tag="lnstd")
    inv = pool.tile([128, 1], F32, name=f"lninv{name}", tag="lninv")
    if skip_mean:
        xm = x_ap
    else:
        mu = pool.tile([128, 1], F32, name=f"lnmu{name}", tag="lnmu")
        xm_t = pool.tile([128, n], F32, name=f"lnxm{name}", tag="lnxm")
        nc.vector.tensor_reduce(mu[:], x_ap, axis=AX.X, op=ALU.add)
        nc.scalar.mul(mu[:], mu[:], inv_n)
        nc.vector.tensor_scalar_sub(xm_t[:], x_ap, mu[:])
        xm = xm_t[:]
    nc.scalar.activation(sq[:], xm, ACTF.Square, accum_out=ssq[:])
    nc.scalar.activation(std[:], ssq[:], ACTF.Sqrt, bias=eps_ap, scale=inv_n)
    nc.vector.reciprocal(inv[:], std[:])
    if out_f32 is not None:
        nc.vector.tensor_scalar_mul(out_f32, xm, inv[:])
    if out_f16 is not None:
        nc.scalar.activation(out_f16, xm, ACTF.Copy, scale=inv[:])
    return xm, inv


def build_program(dbg=False, n_layer=N_LAYER, sim_single=False,
                  stub_sc_ar=False, stub_ym_ar=False, tiny_ar=False):
    if sim_single:
        stub_sc_ar = stub_ym_ar = True
    nc = bacc.Bacc("TRN2", target_bir_lowering=False, debug=False,
                   num_devices=NCORES)
    dbg_o = {}
    if dbg:
        dbg_o["x0"] = nc.dram_tensor("dbg_x0", [T, D], F32, kind="ExternalOutput")
        dbg_o["xs"] = nc.dram_tensor("dbg_xs", [256, T], F32, kind="ExternalOutput")
        dbg_o["qr"] = nc.dram_tensor("dbg_qr", [256, T], F32, kind="ExternalOutput")
        dbg_o["st"] = nc.dram_tensor("dbg_st", [36 * 128, 128], F32, kind="ExternalOutput")
        dbg_o["ykv"] = nc.dram_tensor("dbg_ykv", [T, D], F32, kind="ExternalOutput")
        dbg_o["ym"] = nc.dram_tensor("dbg_ym", [T, D], F32, kind="ExternalOutput")
        dbg_o["x1"] = nc.dram_tensor("dbg_x1", [T, D], F32, kind="ExternalOutput")
        dbg_o["ymp"] = nc.dram_tensor("dbg_ymp", [D, T], F16, kind="ExternalOutput")
        dbg_o["ykvT"] = nc.dram_tensor("dbg_ykvT", [256, T], F32, kind="ExternalOutput")

    # ------------------------------------------------------------- I/O decl
    idx_i = nc.dram_tensor("idx32", [T, 1], F32, kind="ExternalInput")
    embed_i = nc.dram_tensor("embed", [VOCAB, D], F32, kind="ExternalInput")
    enc_i = nc.dram_tensor("enc_sh", [D, NHALF], F16, kind="ExternalInput")
    encv_i = nc.dram_tensor("encv_sh", [D, NHALF], F16, kind="ExternalInput")
    dec_i = nc.dram_tensor("dec_sh", [NHALF, D], F16, kind="ExternalInput")
    lmh_i = nc.dram_tensor("lmh", [D, VOCAB], F16, kind="ExternalInput")
    cos_i = nc.dram_tensor("cos_sh", [NPAIR, T], F16, kind="ExternalInput")
    sin_i = nc.dram_tensor("sin_sh", [NPAIR, T], F16, kind="ExternalInput")
    cmask_i = nc.dram_tensor("cmask", [128, 128], F16, kind="ExternalInput")
    ident_i = nc.dram_tensor("ident", [128, 128], F16, kind="ExternalInput")
    ident32_i = nc.dram_tensor("ident32", [128, 128], F32, kind="ExternalInput")
    out_o = nc.dram_tensor("logits", [T, VOCAB], F32, kind="ExternalOutput")

    pair_groups = [[2 * h, 2 * h + 1] for h in range(NH)]
    all_group = [list(range(NCORES))]

    with tile.TileContext(nc) as tc:
      with (
        tc.tile_pool(name="persist", bufs=1) as pp,
        tc.tile_pool(name="work", bufs=2) as wp,
        tc.tile_pool(name="psW", bufs=2, space="PSUM") as psW,
        tc.tile_pool(name="psAcc", bufs=1, space="PSUM") as psAcc,
        tc.tile_pool(name="dram", bufs=1, space="DRAM") as dp,
      ):
        # ------------------------------------------------- persistent SBUF
        enc_sb = [pp.tile([128, NHALF], F16, name=f"enc{d}", tag=f"enc{d}")
                  for d in range(DC)]
        encv_sb = [pp.tile([128, NHALF], F16, name=f"encv{d}", tag=f"encv{d}")
                   for d in range(DC)]
        QR = [pp.tile([128, T], F16, name=f"qr{i}", tag=f"qr{i}")
              for i in range(NT)]
        ST = [pp.tile([128, 128], F16, name=f"st{i}", tag=f"st{i}")
              for i in range(36)]  # S^T blocks (kb,qb) kb<=qb, fp16, masked
        x_t32 = [pp.tile([128, D], F32, name=f"xt32_{i}", tag=f"xt32_{i}")
                 for i in range(TB)]
        x_t16 = [pp.tile([128, D], F16, name=f"xt16_{i}", tag=f"xt16_{i}")
                 for i in range(TB)]
        x_d16 = [pp.tile([128, T], F16, name=f"xd16_{i}", tag=f"xd16_{i}")
                 for i in range(DC)]
        ykv_t = [pp.tile([128, D], F16, name=f"ykvt{i}", tag=f"ykvt{i}")
                 for i in range(TB)]
        ykvT = [pp.tile([128, T], F16, name=f"ykvT{i}", tag=f"ykvT{i}")
                for i in range(DC)]
        cmask = pp.tile([128, 128], F16, name="cmaskt", tag="cmaskt")
        eps_t = pp.tile([128, 1], F32, name="eps_t", tag="eps_t")
        ident = pp.tile([128, 128], F16, name="identt", tag="identt")
        ident32 = pp.tile([128, 128], F32, name="identt32", tag="identt32")
        lmh_sb = [pp.tile([128, VOCAB], F16, name=f"lmh{d}", tag=f"lmh{d}")
                  for d in range(DC)]

        # ---------------------------------------------------- DRAM buffers
        xs_spill = dp.tile([NHALF, T], F16, name="xs_spill")
        sc_in0 = dp.tile([21 * 128, 128], F16, name="sc_in0")
        sc_out0 = dp.tile([21 * 128, 128], F16, name="sc_out0")
        sc_in1 = dp.tile([15 * 128, 128], F16, name="sc_in1")
        sc_out1 = dp.tile([15 * 128, 128], F16, name="sc_out1")
        tin = dp.tile([128, 128], F16, name="tin")
        touts = [dp.tile([128, 128], F16, name=f"tout{l}", tag=f"tout{l}")
                 for l in range(n_layer)]
        touts8 = [dp.tile([128, 128], F16, name=f"tout8{l}", tag=f"tout8{l}",
                  addr_space="Shared") for l in range(n_layer)]
        ym_in = dp.tile([D, T], F16, name="ym_in")
        ym_outs = [dp.tile([D, T], F16, name=f"ym_out{l}", tag=f"ym_out{l}",
                           addr_space="Shared") for l in range(n_layer)]

        def psw(name, shape=(128, 512), dtype=F32):
            return psW.tile(list(shape), dtype, name=name, tag="ps_w",
                            padded_shape=[128, 512])

        def dbg_dump16(dst_dram, row0, src_ap, w):
            tt = wp.tile([128, w], F32, name="dbgt", tag="dbgt", bufs=1)
            nc.vector.tensor_copy(tt[:], src_ap)
            nc.sync.dma_start(dst_dram[row0:row0 + 128, :], tt[:])

        # ------------------------------------------------------ load consts
        nc.gpsimd.memset(eps_t[:], EPS)
        nc.sync.dma_start(cmask[:], cmask_i[:, :])
        nc.sync.dma_start(ident[:], ident_i[:, :])
        nc.sync.dma_start(ident32[:], ident32_i[:, :])
        for d in range(DC):
            nc.sync.dma_start(enc_sb[d][:], enc_i[128 * d:128 * (d + 1), :])
            nc.sync.dma_start(encv_sb[d][:], encv_i[128 * d:128 * (d + 1), :])
            nc.sync.dma_start(lmh_sb[d][:], lmh_i[128 * d:128 * (d + 1), :])

        # ------------------------------------------------------- embedding
        # E_n = LN(embed) per vocab row; x0 = onehot(idx) @ E_n
        with tc.tile_pool(name="embed", bufs=1) as ep:
            E_n = [ep.tile([128, D], F16, name=f"en{v}", tag=f"en{v}")
                   for v in range(DC)]
            for v in range(DC):
                emb_raw = ep.tile([128, D], F32, name=f"emb_raw{v}",
                                  tag=f"emb_raw{v}")
                nc.sync.dma_start(emb_raw[:], embed_i[128 * v:128 * (v + 1), :])
                _ln_free(nc, wp, emb_raw[:], eps_t[:], out_f16=E_n[v][:],
                         name=f"emb{v}")

            iota_i32 = ep.tile([128, VOCAB], I32, name="iota_i32",
                               tag="iota_i32")
            nc.gpsimd.iota(iota_i32[:], pattern=[[1, VOCAB]], base=0,
                           channel_multiplier=0)
            iota_t = ep.tile([128, VOCAB], F32, name="iota_t", tag="iota_t")
            nc.vector.tensor_copy(iota_t[:], iota_i32[:])
            OHT = [ep.tile([128, T], F16, name=f"oht{v}", tag=f"oht{v}")
                   for v in range(DC)]
            for tb in range(TB):
                idx_col = wp.tile([128, 1], F32, name="idx_col", tag="idx_col")
                nc.sync.dma_start(idx_col[:], idx_i[128 * tb:128 * (tb + 1), :])
                oh_tm = wp.tile([128, VOCAB], F16, name="oh_tm", tag="oh_tm")
                nc.vector.tensor_scalar(oh_tm[:], iota_t[:], idx_col[:], None,
                                        op0=ALU.is_equal)
                for v in range(DC):
                    ps_t = psw(f"ps_tr_oh{tb}_{v}", (128, 128), F16)
                    nc.tensor.transpose(ps_t[:],
                                        oh_tm[:, 128 * v:128 * (v + 1)],
                                        ident[:])
                    nc.scalar.copy(OHT[v][:, 128 * tb:128 * (tb + 1)], ps_t[:])

            for tb in range(TB):
                ps_x = psw(f"ps_x0_{tb}", (128, D))
                for v in range(DC):
                    nc.tensor.matmul(ps_x[:],
                                     OHT[v][:, 128 * tb:128 * (tb + 1)],
                                     E_n[v][:], start=(v == 0),
                                     stop=(v == DC - 1))
                nc.vector.tensor_copy(x_t32[tb][:], ps_x[:])
                nc.scalar.copy(x_t16[tb][:], ps_x[:])
            for d in range(DC):
                for th in range(2):
                    ps_xd = psw(f"ps_xd_{d}_{th}")
                    for v in range(DC):
                        nc.tensor.matmul(
                            ps_xd[:], E_n[v][:, 128 * d:128 * (d + 1)],
                            OHT[v][:, 512 * th:512 * (th + 1)],
                            start=(v == 0), stop=(v == DC - 1))
                    nc.scalar.copy(x_d16[d][:, 512 * th:512 * (th + 1)],
                                   ps_xd[:])

        if dbg:
            for tb in range(TB):
                dbg_dump16(dbg_o["x0"], 128 * tb, x_t32[tb][:], D)

        # ============================================================ layers
        for layer in range(n_layer):
            # ---------------- phase 1a: x_sparse + rope + scores group 0
            acc = {t: psAcc.tile([128, 1024], F32, name=f"{t}_s0_{layer}",
                                 tag=t) for t in ("acc_a", "acc_b", "acc_c")}

            def sc_ap(kb, grp_acc):
                _, tag, off = SC_LAYOUT[kb]
                w = (TB - kb) * 128
                return grp_acc[tag][:, off:off + w]

            for j in range(NJ):
                ct = wp.tile([128, T], F16, name="cos_t", tag="cos_t")
                st_t = wp.tile([128, T], F16, name="sin_t", tag="sin_t")
                nc.sync.dma_start(ct[:], cos_i[128 * j:128 * (j + 1), :])
                nc.sync.dma_start(st_t[:], sin_i[128 * j:128 * (j + 1), :])
                xs_pair = []
                for par in range(2):  # even tile, odd tile
                    nt = 2 * j + par
                    xs_sb = wp.tile([128, T], F16, name="xs_sb", tag="xs_sb")
                    for th in range(2):
                        ps_e = psw(f"ps_enc_{layer}_{nt}_{th}")
                        for d in range(DC):
                            nc.tensor.matmul(
                                ps_e[:],
                                enc_sb[d][:, 128 * nt:128 * (nt + 1)],
                                x_d16[d][:, 512 * th:512 * (th + 1)],
                                start=(d == 0), stop=(d == DC - 1))
                        nc.scalar.activation(xs_sb[:, 512 * th:512 * (th + 1)],
                                             ps_e[:], ACTF.Relu)
                    nc.sync.dma_start(
                        xs_spill[128 * nt:128 * (nt + 1), :], xs_sb[:])
                    xs_pair.append(xs_sb)
                # rope: qr_e = xs_e*c - xs_o*s ; qr_o = xs_o*c + xs_e*s
                xe, xo = xs_pair[0], xs_pair[1]
                qe, qo = QR[2 * j], QR[2 * j + 1]
                p1 = wp.tile([128, T], F16, name="rp1", tag="rp1")
                p2 = wp.tile([128, T], F16, name="rp2", tag="rp2")
                nc.vector.tensor_mul(p1[:], xe[:], ct[:])
                nc.gpsimd.tensor_mul(p2[:], xo[:], st_t[:])
                nc.vector.tensor_sub(qe[:], p1[:], p2[:])
                nc.vector.tensor_mul(p1[:], xo[:], ct[:])
                nc.gpsimd.tensor_mul(p2[:], xe[:], st_t[:])
                nc.vector.tensor_add(qo[:], p1[:], p2[:])
                if dbg and layer == 0 and j == 0:
                    dbg_dump16(dbg_o["xs"], 0, xe[:], T)
                    dbg_dump16(dbg_o["xs"], 128, xo[:], T)
                    dbg_dump16(dbg_o["qr"], 0, qe[:], T)
                    dbg_dump16(dbg_o["qr"], 128, qo[:], T)
                # scores group-0 accumulation for these two n-chunks
                for par in range(2):
                    nt = 2 * j + par
                    first = (j == 0 and par == 0)
                    last = (j == NJ - 1 and par == 1)
                    for kb in range(TB):
                        if SC_LAYOUT[kb][0] != 0:
                            continue
                        dst = sc_ap(kb, acc)
                        w = (TB - kb) * 128
                        for nn in range(0, w, 512):
                            nw = min(512, w - nn)
                            nc.tensor.matmul(
                                dst[:, nn:nn + nw],
                                QR[nt][:, 128 * kb:128 * (kb + 1)],
                                QR[nt][:, 128 * kb + nn:128 * kb + nn + nw],
                                start=first, stop=last)
            # spill score group 0 to DRAM bounce (fp16 via SBUF)
            for kb in range(TB):
                if SC_LAYOUT[kb][0] != 0:
                    continue
                src = sc_ap(kb, acc)
                for qb in range(kb, TB):
                    s_sb = wp.tile([128, 128], F16, name="s_sb", tag="s_sb")
                    nc.scalar.copy(
                        s_sb[:],
                        src[:, 128 * (qb - kb):128 * (qb - kb + 1)])
                    nc.sync.dma_start(
                        sc_in0[128 * _bi(kb, qb):128 * (_bi(kb, qb) + 1), :],
                        s_sb[:])
            # AR of group 0 overlaps with the group-1 matmuls below
            if stub_sc_ar:
                nc.sync.dma_start(sc_out0[:, :], sc_in0[:, :])
                if tiny_ar:
                    nc.gpsimd.collective_compute(
                        "AllReduce", ALU.add, replica_groups=pair_groups,
                        ins=[tin.opt()], outs=[touts[layer].opt()])
            else:
                nc.gpsimd.collective_compute(
                    "AllReduce", ALU.add, replica_groups=pair_groups,
                    ins=[sc_in0.opt()], outs=[sc_out0.opt()])
            for kb in range(TB):
                if SC_LAYOUT[kb][0] != 0:
                    continue
                for qb in range(kb, TB):
                    blk = ST[_bi(kb, qb)]
                    nc.sync.dma_start(
                        blk[:],
                        sc_out0[128 * _bi(kb, qb):128 * (_bi(kb, qb) + 1), :])
                    if qb == kb:
                        nc.vector.tensor_mul(blk[:], blk[:], cmask[:])
            # ---------------- phase 1b: scores group 1 (QR resident)
            acc1 = {t: psAcc.tile([128, 1024], F32, name=f"{t}_s1_{layer}",
                                  tag=t) for t in ("acc_a", "acc_b", "acc_c")}
            for nt in range(NT):
                for kb in range(TB):
                    if SC_LAYOUT[kb][0] != 1:
                        continue
                    dst = sc_ap(kb, acc1)
                    w = (TB - kb) * 128
                    for nn in range(0, w, 512):
                        nw = min(512, w - nn)
                        nc.tensor.matmul(
                            dst[:, nn:nn + nw],
                            QR[nt][:, 128 * kb:128 * (kb + 1)],
                            QR[nt][:, 128 * kb + nn:128 * kb + nn + nw],
                            start=(nt == 0), stop=(nt == NT - 1))
            for kb in range(TB):
                if SC_LAYOUT[kb][0] != 1:
                    continue
                src = sc_ap(kb, acc1)
                for qb in range(kb, TB):
                    s_sb = wp.tile([128, 128], F16, name="s_sb", tag="s_sb")
                    nc.scalar.copy(
                        s_sb[:],
                        src[:, 128 * (qb - kb):128 * (qb - kb + 1)])
                    nc.sync.dma_start(
                        sc_in1[128 * (_bi(kb, qb) - 21):
                               128 * (_bi(kb, qb) - 20), :],
                        s_sb[:])
            # ---------------- scores AllReduce (group 1)
            if stub_sc_ar:
                nc.sync.dma_start(sc_out1[:, :], sc_in1[:, :])
            else:
                nc.gpsimd.collective_compute(
                    "AllReduce", ALU.add, replica_groups=pair_groups,
                    ins=[sc_in1.opt()], outs=[sc_out1.opt()])
            for kb in range(TB):
                if SC_LAYOUT[kb][0] != 1:
                    continue
                for qb in range(kb, TB):
                    blk = ST[_bi(kb, qb)]
                    nc.sync.dma_start(
                        blk[:],
                        sc_out1[128 * (_bi(kb, qb) - 21):
                                128 * (_bi(kb, qb) - 20), :])
                    if qb == kb:
                        nc.vector.tensor_mul(blk[:], blk[:], cmask[:])
            if dbg and layer == 0:
                for kb in range(TB):
                    for qb in range(kb, TB):
                        dbg_dump16(dbg_o["st"], 128 * _bi(kb, qb),
                                   ST[_bi(kb, qb)][:], 128)
            # ---------------- phase 2: yKV + LN + transpose
            # each of the 4 concurrent streams gets its own PSUM bank
            ykv_acc = {}
            for half in range(2):
                for ti, t in enumerate(("acc_a", "acc_b")):
                    ykv_acc[(half, ti)] = psAcc.tile(
                        [128, 1024], F32, name=f"{t}_ykv_{layer}_{half}",
                        tag=t)
            for qb in range(TB):
                ps_y = ykv_acc[(qb // 4, (qb % 4) // 2)][
                    :, 512 * (qb % 2):512 * (qb % 2) + D]
                for kb in range(qb + 1):
                    nc.tensor.matmul(ps_y, ST[_bi(kb, qb)][:], x_t16[kb][:],
                                     start=(kb == 0), stop=(kb == qb))
                _ln_free(nc, wp, ps_y, eps_t[:], out_f16=ykv_t[qb][:],
                         name=f"ykv{qb}")
                for d in range(DC):
                    ps_t = psw(f"ps_tr_ykv{qb}_{d}", (128, 128), F16)
                    nc.tensor.transpose(
                        ps_t[:], ykv_t[qb][:, 128 * d:128 * (d + 1)], ident[:])
                    nc.scalar.copy(ykvT[d][:, 128 * qb:128 * (qb + 1)],
                                   ps_t[:])
            if dbg and layer == 0:
                for qb in range(TB):
                    dbg_dump16(dbg_o["ykv"], 128 * qb, ykv_t[qb][:], D)
                for d in range(DC):
                    dbg_dump16(dbg_o["ykvT"], 128 * d, ykvT[d][:], T)
            # ---------------- phase 3: y_sparse, xy, decoder partials
            # yMLP^T partials [d, t]: one d-half per acc tile; the two
            # 512-wide t-chunks are separate streams in separate banks
            ym_acc = {}
            for half in range(2):
                t = ("acc_a", "acc_b")[half]
                ym_acc[half] = psAcc.tile([128, 1024], F32,
                                          name=f"{t}_ym_{layer}", tag=t)
            for nt in range(NT):
                dec_t = wp.tile([128, D], F16, name="dec_t", tag="dec_t")
                nc.sync.dma_start(dec_t[:], dec_i[128 * nt:128 * (nt + 1), :])
                xs_sb = wp.tile([128, T], F16, name="xs_sb2", tag="xs_sb2")
                nc.sync.dma_start(xs_sb[:],
                                  xs_spill[128 * nt:128 * (nt + 1), :])
                xy = wp.tile([128, T], F16, name="xy", tag="xy")
                for th in range(2):
                    ps_v = psw(f"ps_ysp_{layer}_{nt}_{th}")
                    for d in range(DC):
                        nc.tensor.matmul(
                            ps_v[:], encv_sb[d][:, 128 * nt:128 * (nt + 1)],
                            ykvT[d][:, 512 * th:512 * (th + 1)],
                            start=(d == 0), stop=(d == DC - 1))
                    # xy = relu(ys) * xs  (fused)
                    nc.vector.scalar_tensor_tensor(
                        xy[:, 512 * th:512 * (th + 1)], ps_v[:], 0.0,
                        xs_sb[:, 512 * th:512 * (th + 1)],
                        op0=ALU.max, op1=ALU.mult)
                for dh in range(DC):
                    for thc in range(2):
                        nc.tensor.matmul(
                            ym_acc[dh][:, 512 * thc:512 * (thc + 1)],
                            dec_t[:, 128 * dh:128 * (dh + 1)],
                            xy[:, 512 * thc:512 * (thc + 1)],
                            start=(nt == 0), stop=(nt == NT - 1))
            # ---------------- yMLP AllReduce (sum over heads & halves)
            for dh in range(DC):
                ym_sb = wp.tile([128, T], F16, name="ym_sb", tag="ym_sb",
                                bufs=1)
                nc.vector.tensor_copy(ym_sb[:], ym_acc[dh][:])
                nc.sync.dma_start(ym_in[128 * dh:128 * (dh + 1), :], ym_sb[:])
                if dbg and layer == 0:
                    nc.sync.dma_start(dbg_o["ymp"][128 * dh:128 * (dh + 1), :],
                                      ym_sb[:])
            ym_out = ym_outs[layer]
            if stub_ym_ar:
                nc.sync.dma_start(ym_out[:, :], ym_in[:, :])
                if tiny_ar:
                    nc.gpsimd.collective_compute(
                        "AllReduce", ALU.add, replica_groups=all_group,
                        ins=[tin.opt()], outs=[touts8[layer].opt()])
            else:
                nc.gpsimd.collective_compute(
                    "AllReduce", ALU.add, replica_groups=all_group,
                    ins=[ym_in.opt()], outs=[ym_out.opt()])
            # ---------------- tail: x = LN(x + LN(yMLP))
            um_d = [wp.tile([128, T], F16, name=f"um_d{dh}", tag=f"um_d{dh}",
                            bufs=1)
                    for dh in range(DC)]
            for dh in range(DC):
                nc.sync.dma_start(um_d[dh][:],
                                  ym_out[128 * dh:128 * (dh + 1), :])
            for tb in range(TB):
                u = wp.tile([128, D], F32, name="u_t", tag="u_t")
                for dh in range(DC):
                    ps_t16 = psw(f"ps_tru_{layer}_{tb}_{dh}", (128, 128), F16)
                    nc.tensor.transpose(
                        ps_t16[:], um_d[dh][:, 128 * tb:128 * (tb + 1)],
                        ident[:])
                    nc.scalar.copy(u[:, 128 * dh:128 * (dh + 1)], ps_t16[:])
                if dbg and layer == 0:
                    dbg_dump16(dbg_o["ym"], 128 * tb, u[:], D)
                xm_u, inv_u = _ln_free(nc, wp, u[:], eps_t[:], name=f"u{tb}")
                v = wp.tile([128, D], F32, name="v_t", tag="v_t")
                nc.vector.scalar_tensor_tensor(
                    v[:], xm_u, inv_u[:], x_t32[tb][:],
                    op0=ALU.mult, op1=ALU.add)
                _ln_free(nc, wp, v[:], eps_t[:], out_f32=x_t32[tb][:],
                         out_f16=x_t16[tb][:], skip_mean=True, name=f"v{tb}")
                if dbg and layer == 0:
                    dbg_dump16(dbg_o["x1"], 128 * tb, x_t32[tb][:], D)
                for d in range(DC):
                    ps_t = psw(f"ps_tr_x{layer}_{tb}_{d}", (128, 128), F16)
                    nc.tensor.transpose(
                        ps_t[:], x_t16[tb][:, 128 * d:128 * (d + 1)], ident[:])
                    nc.scalar.copy(x_d16[d][:, 128 * tb:128 * (tb + 1)],
                                   ps_t[:])

        # ------------------------------------------------------- lm head
        for tb in range(TB):
            ps_l = psw(f"ps_lg_{tb}", (128, VOCAB))
            for d in range(DC):
                nc.tensor.matmul(ps_l[:], x_d16[d][:, 128 * tb:128 * (tb + 1)],
                                 lmh_sb[d][:], start=(d == 0),
                                 stop=(d == DC - 1))
            lg_sb = wp.tile([128, VOCAB], F32, name="lg_sb", tag="lg_sb")
            nc.vector.tensor_copy(lg_sb[:], ps_l[:])
            nc.sync.dma_start(out_o[128 * tb:128 * (tb + 1), :], lg_sb[:])

    nc.compile()
    return nc


# ------------------------------------------------------------- host helpers
def _host_tables():
    """cos/sin rope tables in [pair, t] layout, mirroring reference fp32 math."""
    n = np.arange(N, dtype=np.float32)
    q = np.floor(n / 2.0) * 2.0
    freqs = (1.0 / (np.float32(THETA) ** (q / np.float32(N)))
             / np.float32(2.0 * math.pi)).astype(np.float32)
    t = np.arange(T, dtype=np.float32)
    phases = (t[:, None] * freqs[None, :]) % 1.0
    phases = phases * np.float32(2.0 * math.pi)
    cos = np.cos(phases).astype(np.float32)   # [T, N]
    sin = np.sin(phases).astype(np.float32)
    # pair p uses freq of n=2p; table[p, t]
    cos_p = cos[:, 0::2].T.copy()  # [N//2, T]
    sin_p = sin[:, 0::2].T.copy()
    return cos_p, sin_p


def _perm_local():
    """Local latent permutation: position -> (pair index, odd flag)."""
    pos_to_pair = np.empty(NHALF, dtype=np.int64)
    pos_is_odd = np.empty(NHALF, dtype=np.int64)
    for j in range(NJ):
        pr = np.arange(128) + 128 * j
        pos_to_pair[256 * j:256 * j + 128] = pr
        pos_is_odd[256 * j:256 * j + 128] = 0
        pos_to_pair[256 * j + 128:256 * j + 256] = pr
        pos_is_odd[256 * j + 128:256 * j + 256] = 1
    return pos_to_pair, pos_is_odd


_NC_CACHE = {}


def _get_nc():
    if "nc" not in _NC_CACHE:
        _NC_CACHE["nc"] = build_program()
    return _NC_CACHE["nc"]


def prepare_in_maps(idx, embed, encoder, encoder_v, decoder, lm_head):
    idx = np.asarray(idx)
    embed = np.asarray(embed, dtype=np.float32)
    encoder = np.asarray(encoder, dtype=np.float32)
    encoder_v = np.asarray(encoder_v, dtype=np.float32)
    decoder = np.asarray(decoder, dtype=np.float32)
    lm_head = np.asarray(lm_head, dtype=np.float32)

    cos_p, sin_p = _host_tables()
    pos_to_pair, pos_is_odd = _perm_local()

    cmask = (np.arange(128)[:, None] < np.arange(128)[None, :]).astype(np.float16)
    ident = np.eye(128, dtype=np.float16)
    ident32 = np.eye(128, dtype=np.float32)
    idx32 = idx.reshape(T).astype(np.float32).reshape(T, 1)
    lmh16 = lm_head.astype(np.float16)

    in_maps = []
    for c in range(NCORES):
        h, eta = c // 2, c % 2
        pair_g = NPAIR * eta + pos_to_pair          # global pair index
        n_orig = 2 * pair_g + pos_is_odd            # original n within head
        enc_sh = encoder[h][:, n_orig].astype(np.float16)
        encv_sh = encoder_v[h][:, n_orig].astype(np.float16)
        dec_sh = decoder[h * N + n_orig, :].astype(np.float16)
        cos_sh = cos_p[NPAIR * eta:NPAIR * (eta + 1), :].astype(np.float16)
        sin_sh = sin_p[NPAIR * eta:NPAIR * (eta + 1), :].astype(np.float16)
        in_maps.append({
            "idx32": idx32, "embed": embed, "enc_sh": enc_sh,
            "encv_sh": encv_sh, "dec_sh": dec_sh, "lmh": lmh16,
            "cos_sh": cos_sh, "sin_sh": sin_sh, "cmask": cmask,
            "ident": ident, "ident32": ident32,
        })
    return in_maps


def kernel(idx, embed, encoder, encoder_v, decoder, lm_head):
    in_maps = prepare_in_maps(idx, embed, encoder, encoder_v, decoder,
                              lm_head)
    nc = _get_nc()
    res = bass_utils.run_bass_kernel_spmd(nc, in_maps,
                                          core_ids=list(range(NCORES)))
    _NC_CACHE["last_results"] = res
    logits = np.asarray(res.results[0]["logits"], dtype=np.float32)
    return logits.reshape(1, T, VOCAB)



# revision 13
# speedup vs baseline: 1.2647x; 1.2647x over previous
"""Trainium2 Bass kernel for nn_BDH_6313601925221 (sparse_attention).

Model (reference.py):
  x = LN(embed[idx])                                   (B=1, T=1024, D=256)
  repeat 6 layers (shared weights):
    x_sparse = relu(einsum('btd,hdn->bhtn', x, encoder))   N=8192, NH=4
    QR       = rope(x_sparse)                              interleaved-pair rotation
    scores   = einsum('bhtn,bhsn->bhts', QR, QR) * strict_causal
    yKV      = LN(einsum('bhts,bsd->bhtd', scores, x))
    y_sparse = relu(einsum('bhtd,hdn->bhtn', yKV, encoder_v))
    yMLP     = (x_sparse*y_sparse).transpose -> (T, NH*N) @ decoder
    x        = LN(x + LN(yMLP))
  logits = x @ lm_head

Distribution (8 cores): core c = (head h=c//2, latent-half eta=c%2).
Each core computes the encoder/rope/scores path over its 4096 latent dims.
Partial scores stay on-core (PSUM -> SBUF fp16); the contraction over the
latent halves is done by AllReducing the PARTIAL yKV within the head pair
(2 chunks of 4 token-blocks, pipelined against the remaining score strips).
y_sparse/xy/decoder run over the local latent half for all tokens; the
yMLP^T partials are AllReduced over all 8 ranks in 2 token-half chunks so
each AR hides under the other half's matmuls.

x is kept d-major ([128, 512] tiles per (dh, th)); the inter-layer LNs
compute stats along the partition (d) axis via ones-matmuls + PE
row-broadcasts, so the next layer's encoder matmuls start straight off the
AR result with no transpose chain. Token-major x (yKV rhs) is produced
lazily by 16 PE transposes.

QR storage (SBUF pressure): persistent QR_t (t-cols 512:1024) + QR_k3
(cols 384:512); the head (cols 0:384) is written IN PLACE into the xs
th0-half tiles (xs is spilled to DRAM before rope overwrites it). Score
strips therefore accumulate from up to 4 source chunks; within a PSUM
bank only the first chunk's nt==0 matmul uses start=True (clearing the
bank's has_written bits), later chunks' first matmuls use start=False and
overwrite-by-virgin-bit.

Layouts: latent dim N is host-permuted so rope pairs are de-interleaved:
local tile 2j = even pair members, 2j+1 = odd. Inner products over N and
the decoder contraction are invariant to this permutation (weights are
permuted to match).

PSUM (8 banks): psA tags pa/pb/pc (2 banks each) carry score-strip and
yKV/yMLP accumulations; psW (2 banks, bufs=2) carries transient matmul
outputs. The tail LN stats/broadcasts use pa generations so they never
block the ps_w FIFO that phase 3 depends on.
"""

import math
import sys

import numpy as np

for _p in ("/opt/trn_rl_repo",):
    if _p not in sys.path:
        sys.path.insert(0, _p)

import concourse.bass as bass
import concourse.mybir as mybir
import concourse.tile as tile
from concourse import bacc
from concourse import bass_utils

# ---------------------------------------------------------------- constants
D = 256
NH = 4
N = 8192
T = 1024
N_LAYER = 6
VOCAB = 256
THETA = 2 ** 16
EPS = 1e-5
NCORES = 8

NHALF = N // 2          # 4096 latent dims per core
NPAIR = NHALF // 2      # 2048 rope pairs per core
NT = NHALF // 128       # 32 local n-tiles of 128
NJ = NT // 2            # 16 pair-blocks (tile 2j = evens, 2j+1 = odds)
TB = T // 128           # 8 token blocks
DC = D // 128           # 2 d-chunks

F16 = mybir.dt.float16
F32 = mybir.dt.float32
I32 = mybir.dt.int32
AX = mybir.AxisListType
ALU = mybir.AluOpType
ACTF = mybir.ActivationFunctionType

INV_D = 1.0 / D


def _ln_free(nc, pool, x_ap, eps_ap, out_f16=None, n=None, name=""):
    """LayerNorm along the free dim of a [128, n] tile (per-partition stats)."""
    n = n if n is not None else x_ap.shape[-1]
    inv_n = 1.0 / n
    sq = pool.tile([128, n], F32, name=f"lnsq{name}", tag="lnsq")
    ssq = pool.tile([128, 1], F32, name=f"lnssq{name}", tag="lnssq")
    std = pool.tile([128, 1], F32, name=f"lnstd{name}", tag="lnstd")
    inv = pool.tile([128, 1], F32, name=f"lninv{name}", tag="lninv")
    mu = pool.tile([128, 1], F32, name=f"lnmu{name}", tag="lnmu")
    xm_t = pool.tile([128, n], F32, name=f"lnxm{name}", tag="lnxm")
    nc.vector.tensor_reduce(mu[:], x_ap, axis=AX.X, op=ALU.add)
    nc.scalar.mul(mu[:], mu[:], inv_n)
    nc.vector.tensor_scalar_sub(xm_t[:], x_ap, mu[:])
    xm = xm_t[:]
    nc.scalar.activation(sq[:], xm, ACTF.Square, accum_out=ssq[:])
    nc.scalar.activation(std[:], ssq[:], ACTF.Sqrt, bias=eps_ap, scale=inv_n)
    nc.vector.reciprocal(inv[:], std[:])
    if out_f16 is not None:
        nc.scalar.activation(out_f16, xm, ACTF.Copy, scale=inv[:])
    return xm, inv


def build_program(n_layer=N_LAYER):
    nc = bacc.Bacc("TRN2", target_bir_lowering=False, debug=False,
                   num_devices=NCORES)

    # ------------------------------------------------------------- I/O decl
    idx_i = nc.dram_tensor("idx32", [T, 1], F32, kind="ExternalInput")
    embed_i = nc.dram_tensor("embed", [VOCAB, D], F32, kind="ExternalInput")
    enc_i = nc.dram_tensor("enc_sh", [D, NHALF], F16, kind="ExternalInput")
    encv_i = nc.dram_tensor("encv_sh", [D, NHALF], F16, kind="ExternalInput")
    dec_i = nc.dram_tensor("dec_sh", [NHALF, D], F16, kind="ExternalInput")
    lmh_i = nc.dram_tensor("lmh", [D, VOCAB], F16, kind="ExternalInput")
    cos_i = nc.dram_tensor("cos_sh", [NPAIR, T], F16, kind="ExternalInput")
    sin_i = nc.dram_tensor("sin_sh", [NPAIR, T], F16, kind="ExternalInput")
    cmask_i = nc.dram_tensor("cmask", [128, 128], F16, kind="ExternalInput")
    ident_i = nc.dram_tensor("ident", [128, 128], F16, kind="ExternalInput")
    out_o = nc.dram_tensor("logits", [T, VOCAB], F32, kind="ExternalOutput")

    pair_groups = [[2 * h, 2 * h + 1] for h in range(NH)]
    all_group = [list(range(NCORES))]

    with tile.TileContext(nc) as tc:
      with (
        tc.tile_pool(name="persist", bufs=1) as pp,
        tc.tile_pool(name="work", bufs=2) as wp,
        tc.tile_pool(name="psW", bufs=2, space="PSUM") as psW,
        tc.tile_pool(name="psA", bufs=1, space="PSUM") as psA,
        tc.tile_pool(name="dram", bufs=1, space="DRAM") as dp,
      ):
        # ------------------------------------------------- persistent SBUF
        enc_sb = [pp.tile([128, NHALF], F16, name=f"enc{d}", tag=f"enc{d}")
                  for d in range(DC)]
        encv_sb = [pp.tile([128, NHALF], F16, name=f"encv{d}", tag=f"encv{d}")
                   for d in range(DC)]
        # QR tail (t-cols 512:1024) and k3 block (cols 384:512)
        QR_t = [pp.tile([128, 512], F16, name=f"qrt{i}", tag=f"qrt{i}")
                for i in range(NT)]
        QR_k3 = [pp.tile([128, 128], F16, name=f"qrk{i}", tag=f"qrk{i}")
                 for i in range(NT)]
        # score strips S^T[kb]: [s=128, (TB-kb)*128] fp16, masked on diag blk
        STS = [pp.tile([128, (TB - kb) * 128], F16, name=f"sts{kb}",
                       tag=f"sts{kb}") for kb in range(TB)]
        # x in d-major: per (dh, th) [128, 512] fp16 (residual carried fp16)
        x_d16 = [[pp.tile([128, 512], F16, name=f"xd16_{dh}_{th}",
                          tag=f"xd16_{dh}_{th}") for th in range(2)]
                 for dh in range(DC)]
        x_t16 = [pp.tile([128, D], F16, name=f"xt16_{i}", tag=f"xt16_{i}")
                 for i in range(TB)]
        ykvT = [[pp.tile([128, 512], F16, name=f"ykvT{d}_{th}",
                         tag=f"ykvT{d}_{th}") for th in range(2)]
                for d in range(DC)]
        # xs th0 halves for nt 0..15; after rope their cols [0:384] hold the
        # QR head (stationaries + moving chunks for score strips kb 0..2)
        xs_h0 = [pp.tile([128, 512], F16, name=f"xsh0_{i}", tag=f"xsh0_{i}")
                 for i in range(16)]
        cmask = pp.tile([128, 128], F16, name="cmaskt", tag="cmaskt")
        eps_t = pp.tile([128, 1], F32, name="eps_t", tag="eps_t")
        ident = pp.tile([128, 128], F16, name="identt", tag="identt")
        ones_c = pp.tile([128, 1], F16, name="ones_c", tag="ones_c")
        ones_r = pp.tile([1, 128], F16, name="ones_r", tag="ones_r")
        lmh_sb = [pp.tile([128, VOCAB], F16, name=f"lmh{d}", tag=f"lmh{d}")
                  for d in range(DC)]

        # ---------------------------------------------------- DRAM buffers
        xs_spill = dp.tile([NHALF, T], F16, name="xs_spill")
        ykv_in = [dp.tile([512, D], F16, name=f"ykv_in{c}") for c in range(2)]
        ykv_out = [[dp.tile([512, D], F16, name=f"ykv_out{l}_{c}")
                    for c in range(2)]
                   for l in range(n_layer)]
        ym_in = [dp.tile([D, 512], F16, name=f"ym_in{c}") for c in range(2)]
        ym_out = [[dp.tile([D, 512], F16, name=f"ym_out{l}_{c}",
                           addr_space="Shared") for c in range(2)]
                  for l in range(n_layer)]

        def psw(name, shape=(128, 512), dtype=F32):
            return psW.tile(list(shape), dtype, name=name, tag="ps_w",
                            padded_shape=[128, 512])

        def pat(tag, name, dtype=F32):
            return psA.tile([128, 1024], dtype, name=name, tag=tag)

        # ------------------------------------------------------ load consts
        nc.gpsimd.memset(eps_t[:], EPS)
        nc.gpsimd.memset(ones_c[:], 1.0)
        nc.gpsimd.memset(ones_r[:], 1.0)
        nc.sync.dma_start(cmask[:], cmask_i[:, :])
        nc.sync.dma_start(ident[:], ident_i[:, :])
        for d in range(DC):
            nc.sync.dma_start(enc_sb[d][:], enc_i[128 * d:128 * (d + 1), :])
            nc.sync.dma_start(encv_sb[d][:], encv_i[128 * d:128 * (d + 1), :])
            nc.sync.dma_start(lmh_sb[d][:], lmh_i[128 * d:128 * (d + 1), :])

        # ------------------------------------------------------- embedding
        # E_n = LN(embed) per vocab row; x0 = onehot(idx) @ E_n
        with tc.tile_pool(name="embed", bufs=1) as ep:
            E_n = [ep.tile([128, D], F16, name=f"en{v}", tag=f"en{v}")
                   for v in range(DC)]
            for v in range(DC):
                emb_raw = ep.tile([128, D], F32, name=f"emb_raw{v}",
                                  tag=f"emb_raw{v}")
                nc.sync.dma_start(emb_raw[:], embed_i[128 * v:128 * (v + 1), :])
                _ln_free(nc, wp, emb_raw[:], eps_t[:], out_f16=E_n[v][:],
                         name=f"emb{v}")

            iota_i32 = ep.tile([128, VOCAB], I32, name="iota_i32",
                               tag="iota_i32")
            nc.gpsimd.iota(iota_i32[:], pattern=[[1, VOCAB]], base=0,
                           channel_multiplier=0)
            iota_t = ep.tile([128, VOCAB], F32, name="iota_t", tag="iota_t")
            nc.vector.tensor_copy(iota_t[:], iota_i32[:])
            OHT = [ep.tile([128, T], F16, name=f"oht{v}", tag=f"oht{v}")
                   for v in range(DC)]
            for tb in range(TB):
                idx_col = ep.tile([128, 1], F32, name="idx_col",
                                  tag="idx_col", bufs=2)
                nc.sync.dma_start(idx_col[:], idx_i[128 * tb:128 * (tb + 1), :])
                oh_tm = ep.tile([128, VOCAB], F16, name="oh_tm", tag="oh_tm",
                                bufs=2)
                nc.vector.tensor_scalar(oh_tm[:], iota_t[:], idx_col[:], None,
                                        op0=ALU.is_equal)
                for v in range(DC):
                    ps_t = psw(f"ps_tr_oh{tb}_{v}", (128, 128), F16)
                    nc.tensor.transpose(ps_t[:],
                                        oh_tm[:, 128 * v:128 * (v + 1)],
                                        ident[:])
                    nc.scalar.copy(OHT[v][:, 128 * tb:128 * (tb + 1)], ps_t[:])

            # token-major x0 (for yKV rhs)
            for tb in range(TB):
                ps_x = psw(f"ps_x0_{tb}", (128, D))
                for v in range(DC):
                    nc.tensor.matmul(ps_x[:],
                                     OHT[v][:, 128 * tb:128 * (tb + 1)],
                                     E_n[v][:], start=(v == 0),
                                     stop=(v == DC - 1))
                nc.scalar.copy(x_t16[tb][:], ps_x[:])
            # d-major x0
            for d in range(DC):
                for th in range(2):
                    ps_xd = psw(f"ps_xd_{d}_{th}")
                    for v in range(DC):
                        nc.tensor.matmul(
                            ps_xd[:], E_n[v][:, 128 * d:128 * (d + 1)],
                            OHT[v][:, 512 * th:512 * (th + 1)],
                            start=(v == 0), stop=(v == DC - 1))
                    nc.scalar.copy(x_d16[d][th][:], ps_xd[:])

        # ------------------------------------------------------ helpers
        def enc_mm(nt, th, xs_dst_ap, relu_on_vec):
            """xs[nt][:, th] = relu(enc^T x) for one (nt, th)."""
            ps_e = psw(f"ps_enc_{nt}_{th}")
            for d in range(DC):
                nc.tensor.matmul(
                    ps_e[:], enc_sb[d][:, 128 * nt:128 * (nt + 1)],
                    x_d16[d][th][:], start=(d == 0), stop=(d == DC - 1))
            if relu_on_vec:
                nc.vector.tensor_scalar(xs_dst_ap, ps_e[:], 0.0, None,
                                        op0=ALU.max)
            else:
                nc.scalar.activation(xs_dst_ap, ps_e[:], ACTF.Relu)

        def rope_half(j, th, xe_t, xo_t, ct_ap, st_ap, g_ops):
            """Rope for pair-block j, token half th.

            th==1: writes QR_t[2j]/QR_t[2j+1] (full [128,512] ops).
            th==0: writes the head back IN PLACE into xe_t/xo_t cols [0:384]
                   and the k3 block into QR_k3 (reads complete first).
            xe_t/xo_t are [128,512] tiles (th-half of xs for nt=2j, 2j+1).
            g_ops: subset of {'p2','p4'} to run on gpsimd."""
            p1 = wp.tile([128, 512], F16, name="rp1", tag="rp1", bufs=1)
            p2 = wp.tile([128, 512], F16, name="rp2", tag="rp2", bufs=2)
            p3 = wp.tile([128, 512], F16, name="rp3", tag="rp3", bufs=1)
            p4 = wp.tile([128, 512], F16, name="rp4", tag="rp4", bufs=2)
            nc.vector.tensor_mul(p1[:], xe_t[:], ct_ap)
            eng = nc.gpsimd if 'p2' in g_ops else nc.vector
            eng.tensor_mul(p2[:], xo_t[:], st_ap)
            nc.vector.tensor_mul(p3[:], xo_t[:], ct_ap)
            eng = nc.gpsimd if 'p4' in g_ops else nc.vector
            eng.tensor_mul(p4[:], xe_t[:], st_ap)
            if th == 1:
                nc.vector.tensor_sub(QR_t[2 * j][:], p1[:], p2[:])
                nc.vector.tensor_add(QR_t[2 * j + 1][:], p3[:], p4[:])
            else:
                nc.vector.tensor_sub(xe_t[:, 0:384], p1[:, 0:384],
                                     p2[:, 0:384])
                nc.vector.tensor_sub(QR_k3[2 * j][:], p1[:, 384:512],
                                     p2[:, 384:512])
                nc.vector.tensor_add(xo_t[:, 0:384], p3[:, 0:384],
                                     p4[:, 0:384])
                nc.vector.tensor_add(QR_k3[2 * j + 1][:], p3[:, 384:512],
                                     p4[:, 384:512])

        def load_costab(j, th):
            ct = wp.tile([128, 512], F16, name=f"cos{th}", tag=f"cos{th}",
                         bufs=3)
            st = wp.tile([128, 512], F16, name=f"sin{th}", tag=f"sin{th}",
                         bufs=3)
            cols = slice(512 * th, 512 * (th + 1))
            nc.sync.dma_start(ct[:], cos_i[128 * j:128 * (j + 1), cols])
            nc.sync.dma_start(st[:], sin_i[128 * j:128 * (j + 1), cols])
            return ct, st

        def strip_chunks(kb, head_ap, nt):
            """(src_ap, strip col offset, width) chunks for strip kb of nt.

            Strip kb covers t-cols [128*kb, 1024) => widths (TB-kb)*128.
            Sources: head (xs tile cols [128kb:384]), QR_k3, QR_t halves
            split at the psum bank boundary (strip col 512)."""
            out = []
            off = 0
            if kb <= 2:
                w = 384 - 128 * kb
                out.append((head_ap[:, 128 * kb:384], off, w))
                off += w
            if kb <= 3:
                out.append((QR_k3[nt][:], off, 128))
                off += 128
                # QR_t part(s), split at strip col 512
                b = 512 - off
                if b > 0:
                    out.append((QR_t[nt][:, 0:b], off, b))
                out.append((QR_t[nt][:, b:512], 512, 512 - b))
            else:
                c0 = 128 * (kb - 4)
                out.append((QR_t[nt][:, c0:512], 0, 512 - c0))
            return out

        def strip_station(kb, head_ap, nt):
            if kb <= 2:
                return head_ap[:, 128 * kb:128 * (kb + 1)]
            if kb == 3:
                return QR_k3[nt][:]
            return QR_t[nt][:, 128 * (kb - 4):128 * (kb - 3)]

        def strip_nt_mms(kb, dst_tile, head_ap, nt):
            """All matmuls of strip kb for one nt.

            A strip's chunks can share a psum bank, so per bank: start=True
            only on the first chunk at nt==0 (clears stale has_written; later
            chunks' first matmuls overwrite via their virgin bits), and
            stop=True only on the bank's last chunk at nt==NT-1 (stop clears
            the sim's group-started flag)."""
            stat = strip_station(kb, head_ap, nt)
            chunks = strip_chunks(kb, head_ap, nt)
            first_in_bank = {}
            last_in_bank = {}
            for i, (_, off, _) in enumerate(chunks):
                bank = off // 512
                first_in_bank.setdefault(bank, i)
                last_in_bank[bank] = i
            for i, (src_ap, off, w) in enumerate(chunks):
                bank = off // 512
                nc.tensor.matmul(
                    dst_tile[:, off:off + w], stat, src_ap,
                    start=(nt == 0 and first_in_bank[bank] == i),
                    stop=(nt == NT - 1 and last_in_bank[bank] == i))

        def strip_copy(kb, src_tile):
            """PSUM score strip kb -> STS[kb] fp16, diag block masked."""
            w = (TB - kb) * 128
            nc.vector.tensor_mul(STS[kb][:, 0:128], src_tile[:, 0:128],
                                 cmask[:])
            if w > 128:
                nc.scalar.copy(STS[kb][:, 128:w], src_tile[:, 128:w])

        def ykv_mms(qb, region_ap):
            """Partial yKV for token block qb into a psum region [128, 256]."""
            for kb in range(qb + 1):
                nc.tensor.matmul(
                    region_ap,
                    STS[kb][:, 128 * (qb - kb):128 * (qb - kb) + 128],
                    x_t16[kb][:], start=(kb == 0), stop=(kb == qb))

        def ykv_store(qb, region_ap, chunk):
            # yKV partials are pre-LN and can exceed fp16 range; LN is
            # scale-invariant, so store them scaled down by 1/256.
            stg = wp.tile([128, D], F16, name="ykvst", tag="ykvst")
            nc.scalar.activation(stg[:], region_ap, ACTF.Copy,
                                 scale=1.0 / 256.0)
            q = qb % 4
            nc.sync.dma_start(ykv_in[chunk][128 * q:128 * (q + 1), :], stg[:])

        def ykv_process(layer, chunk, tr_tile=None):
            """Load AR'd yKV chunk, LN, transpose into ykvT[:][chunk].

            tr_tile: optional F16 psum tile for the transposes (so they do
            not share the ps_w FIFO with concurrently-running phase-3 tiles).
            """
            for q in range(4):
                qb = 4 * chunk + q
                ld = wp.tile([128, D], F16, name="ykvld", tag="ykvld")
                nc.sync.dma_start(
                    ld[:], ykv_out[layer][chunk][128 * q:128 * (q + 1), :])
                yt = wp.tile([128, D], F16, name="ykvt", tag="ykvt", bufs=3)
                _ln_free(nc, wp, ld[:], eps_t[:], out_f16=yt[:],
                         name=f"ykv{qb}")
                for d in range(DC):
                    if tr_tile is None:
                        ps_t = psw(f"ps_tr_ykv{qb}_{d}", (128, 128), F16)[:]
                    else:
                        c0 = 128 * (2 * q + d)
                        ps_t = tr_tile[:, c0:c0 + 128]
                    nc.tensor.transpose(
                        ps_t, yt[:, 128 * d:128 * (d + 1)], ident[:])
                    nc.scalar.copy(ykvT[d][chunk][:, 128 * q:128 * (q + 1)],
                                   ps_t)

        def xt_transpose(tb):
            """x_t16[tb] from d-major x (2 PE transposes + copies)."""
            th, tc_ = divmod(tb, 4)
            for d in range(DC):
                ps_t = psw(f"ps_tr_x{tb}_{d}", (128, 128), F16)
                nc.tensor.transpose(
                    ps_t[:], x_d16[d][th][:, 128 * tc_:128 * (tc_ + 1)],
                    ident[:])
                nc.scalar.copy(x_t16[tb][:, 128 * d:128 * (d + 1)], ps_t[:])

        def phase3_nt(layer, nt, th, ym_acc):
            """y_sparse, xy, decoder partials for one (nt, th)."""
            dec_t = wp.tile([128, D], F16, name="dec_t", tag="dec_t", bufs=3)
            nc.sync.dma_start(dec_t[:], dec_i[128 * nt:128 * (nt + 1), :])
            xs_t = wp.tile([128, 512], F16, name="xs3", tag="xs3", bufs=3)
            nc.sync.dma_start(
                xs_t[:],
                xs_spill[128 * nt:128 * (nt + 1), 512 * th:512 * (th + 1)])
            ps_v = psw(f"ps_ysp_{layer}_{nt}_{th}")
            for d in range(DC):
                nc.tensor.matmul(
                    ps_v[:], encv_sb[d][:, 128 * nt:128 * (nt + 1)],
                    ykvT[d][th][:], start=(d == 0), stop=(d == DC - 1))
            xy = wp.tile([128, 512], F16, name="xy", tag="xy", bufs=2)
            nc.vector.scalar_tensor_tensor(
                xy[:], ps_v[:], 0.0, xs_t[:], op0=ALU.max, op1=ALU.mult)
            for dh in range(DC):
                nc.tensor.matmul(
                    ym_acc[:, 512 * dh:512 * dh + 512],
                    dec_t[:, 128 * dh:128 * (dh + 1)],
                    xy[:], start=(nt == 0), stop=(nt == NT - 1))

        def ym_store(th, ym_acc):
            for dh in range(DC):
                stg = wp.tile([128, 512], F16, name="ymst", tag="ymst")
                nc.scalar.copy(stg[:], ym_acc[:, 512 * dh:512 * dh + 512])
                nc.sync.dma_start(
                    ym_in[th][128 * dh:128 * (dh + 1), :], stg[:])

        def bcast_stats(um_list, name):
            """Per-column LN affine (a, b) from the partition-dim stats of
            the two [128,512] tiles in um_list; returns psum broadcasts.

            Uses pa psum generations (not ps_w) so the stats chain never
            blocks the ps_w FIFO that concurrently running phases depend on.
            """
            rows = pat("pa", f"r_{name}")
            mu_ps = rows[0:1, 0:512]
            s2_ps = rows[0:1, 512:1024]
            sq = []
            for dh in range(DC):
                nc.tensor.matmul(mu_ps, ones_c[:], um_list[dh],
                                 start=(dh == 0), stop=(dh == DC - 1))
                sqt = wp.tile([128, 512], F16, name="tsq", tag="tsq")
                nc.vector.tensor_mul(sqt[:], um_list[dh], um_list[dh])
                sq.append(sqt)
            for dh in range(DC):
                nc.tensor.matmul(s2_ps, ones_c[:], sq[dh][:],
                                 start=(dh == 0), stop=(dh == DC - 1))
            m_row = wp.tile([1, 512], F32, name="mrow", tag="mrow")
            nc.vector.tensor_scalar(m_row[:], mu_ps, INV_D, None,
                                    op0=ALU.mult)
            mm2 = wp.tile([1, 512], F32, name="mm2", tag="mm2")
            nc.vector.tensor_mul(mm2[:], m_row[:], m_row[:])
            var_r = wp.tile([1, 512], F32, name="varr", tag="varr")
            nc.vector.scalar_tensor_tensor(var_r[:], s2_ps, INV_D,
                                           mm2[:], op0=ALU.mult,
                                           op1=ALU.subtract)
            std_r = wp.tile([1, 512], F32, name="stdr", tag="stdr")
            nc.scalar.activation(std_r[:], var_r[:], ACTF.Sqrt,
                                 bias=eps_t[0:1, :])
            inv_r = wp.tile([1, 512], F32, name="invr", tag="invr")
            nc.vector.reciprocal(inv_r[:], std_r[:])
            a_row = wp.tile([1, 512], F16, name="arow", tag="arow")
            nc.vector.tensor_copy(a_row[:], inv_r[:])
            b_row = wp.tile([1, 512], F16, name="brow", tag="brow")
            nc.vector.scalar_tensor_tensor(b_row[:], m_row[:], -1.0, inv_r[:],
                                           op0=ALU.mult, op1=ALU.mult)
            bc = pat("pa", f"b_{name}")
            nc.tensor.matmul(bc[:, 0:512], ones_r[:], a_row[:], start=True,
                             stop=True)
            nc.tensor.matmul(bc[:, 512:1024], ones_r[:], b_row[:], start=True,
                             stop=True)
            return bc[:, 0:512], bc[:, 512:1024]

        def tail_chunk(layer, th):
            """x[:, th] = LN(x + LN(yMLP[:, th])) in d-major (fp16 residual)."""
            um = []
            for dh in range(DC):
                u = wp.tile([128, 512], F16, name=f"um{dh}", tag=f"um{dh}",
                            bufs=1)
                nc.sync.dma_start(
                    u[:], ym_out[layer][th][128 * dh:128 * (dh + 1), :])
                um.append(u)
            a_b, b_b = bcast_stats([u[:] for u in um], f"t1_{layer}_{th}")
            v16 = []
            for dh in range(DC):
                t1 = wp.tile([128, 512], F32, name="tz1", tag="tz1", bufs=1)
                nc.vector.tensor_mul(t1[:], um[dh][:], a_b)
                t2 = wp.tile([128, 512], F32, name="tz2", tag="tz2", bufs=1)
                nc.vector.tensor_add(t2[:], t1[:], b_b)
                v = wp.tile([128, 512], F16, name="tv16", tag=f"tv16{dh}",
                            bufs=1)
                nc.vector.tensor_add(v[:], t2[:], x_d16[dh][th][:])
                v16.append(v)
            a2_b, b2_b = bcast_stats([v[:] for v in v16], f"t2_{layer}_{th}")
            for dh in range(DC):
                u1 = wp.tile([128, 512], F32, name="tu1", tag="tu1", bufs=1)
                nc.vector.tensor_mul(u1[:], v16[dh][:], a2_b)
                nc.vector.tensor_add(x_d16[dh][th][:], u1[:], b2_b)

        # ============================================================ layers
        for layer in range(n_layer):
            # ---------------- pass A: enc waves + rope + score group 0
            # wave A: th0 for nt 0..15 (x th0 ready from tail_chunk(...,0))
            if layer > 0:
                for tb in range(4):
                    xt_transpose(tb)
            for nt in range(16):
                enc_mm(nt, 0, xs_h0[nt][:], relu_on_vec=(nt % 2 == 1))
                nc.sync.dma_start(
                    xs_spill[128 * nt:128 * (nt + 1), 0:512], xs_h0[nt][:])
                if nt % 2 == 1:
                    j = nt // 2
                    ct, st = load_costab(j, 0)
                    rope_half(j, 0, xs_h0[nt - 1], xs_h0[nt],
                              ct[:], st[:], g_ops={'p2', 'p4'})
            # tail th1 of previous layer (vector chain; waits ym AR chunk 1)
            if layer > 0:
                tail_chunk(layer - 1, 1)
                for tb in range(4, 8):
                    xt_transpose(tb)
            # wave B: th1 for nt 0..15; rope th1; scores group0 skew 2
            acc = {0: pat("pa", f"sc0_{layer}"),
                   1: pat("pb", f"sc1_{layer}"),
                   2: pat("pc", f"sc2_{layer}")}
            head_of = {}

            def scores_g0(nt):
                for kb in range(3):
                    strip_nt_mms(kb, acc[kb], head_of[nt][:], nt)

            for nt in range(16):
                head_of[nt] = xs_h0[nt]
                xs1 = wp.tile([128, 512], F16, name="xsh1", tag="xsh1",
                              bufs=4)
                enc_mm(nt, 1, xs1[:], relu_on_vec=(nt % 2 == 0))
                nc.sync.dma_start(
                    xs_spill[128 * nt:128 * (nt + 1), 512:1024], xs1[:])
                if nt % 2 == 0:
                    xs1_prev = xs1
                else:
                    j = nt // 2
                    ct, st = load_costab(j, 1)
                    rope_half(j, 1, xs1_prev, xs1, ct[:], st[:],
                              g_ops={'p2'})
                    if j >= 2:
                        scores_g0(2 * (j - 2))
                        scores_g0(2 * (j - 2) + 1)
            # nt 16..31: both th, rope both halves, scores skew 2
            for nt in range(16, 32):
                x0t = wp.tile([128, 512], F16, name="xs30", tag="xs30",
                              bufs=6)
                head_of[nt] = x0t
                enc_mm(nt, 0, x0t[:], relu_on_vec=False)
                nc.sync.dma_start(
                    xs_spill[128 * nt:128 * (nt + 1), 0:512], x0t[:])
                x1t = wp.tile([128, 512], F16, name="xs31", tag="xs31",
                              bufs=4)
                enc_mm(nt, 1, x1t[:], relu_on_vec=True)
                nc.sync.dma_start(
                    xs_spill[128 * nt:128 * (nt + 1), 512:1024], x1t[:])
                if nt % 2 == 0:
                    xs_prev = (x0t, x1t)
                else:
                    j = nt // 2
                    ct, st = load_costab(j, 1)
                    rope_half(j, 1, xs_prev[1], x1t, ct[:], st[:],
                              g_ops={'p2'})
                    ct, st = load_costab(j, 0)
                    rope_half(j, 0, xs_prev[0], x0t, ct[:], st[:],
                              g_ops={'p2'})
                    scores_g0(2 * (j - 2))
                    scores_g0(2 * (j - 2) + 1)
            for nt in range(28, 32):
                scores_g0(nt)

            # ---------------- pass B: strips kb 3..7 + partial yKV + AR
            for kb in range(3):
                strip_copy(kb, acc[kb])
            # pc freed by strip_copy(2) -> yKV pair tiles cycle through pc
            pcT = pat("pc", f"ykv01_{layer}")
            ykv_mms(0, pcT[:, 0:256])
            ykv_mms(1, pcT[:, 512:768])
            ykv_store(0, pcT[:, 0:256], 0)
            ykv_store(1, pcT[:, 512:768], 0)

            strip_tag = {3: "pa", 4: "pb", 5: "pa", 6: "pb", 7: "pa"}

            def strip_mms(kb):
                stile = pat(strip_tag[kb], f"st_{layer}_{kb}")
                for nt in range(NT):
                    strip_nt_mms(kb, stile, head_of[nt][:], nt)
                strip_copy(kb, stile)

            strip_mms(3)
            pcT = pat("pc", f"ykv23_{layer}")
            ykv_mms(2, pcT[:, 0:256])
            ykv_mms(3, pcT[:, 512:768])
            ykv_store(2, pcT[:, 0:256], 0)
            ykv_store(3, pcT[:, 512:768], 0)
            nc.gpsimd.collective_compute(
                "AllReduce", ALU.add, replica_groups=pair_groups,
                ins=[ykv_in[0].opt()], outs=[ykv_out[layer][0].opt()])

            strip_mms(4)
            pcT = pat("pc", f"ykv45_{layer}")
            ykv_mms(4, pcT[:, 0:256])
            strip_mms(5)
            ykv_mms(5, pcT[:, 512:768])
            ykv_store(4, pcT[:, 0:256], 1)
            ykv_store(5, pcT[:, 512:768], 1)
            strip_mms(6)
            pcT = pat("pc", f"ykv67_{layer}")
            ykv_mms(6, pcT[:, 0:256])
            # yKV chunk 0 is back around here: LN + transpose into ykvT th0
            ykv_process(layer, 0)
            strip_mms(7)
            ykv_mms(7, pcT[:, 512:768])
            ykv_store(6, pcT[:, 0:256], 1)
            ykv_store(7, pcT[:, 512:768], 1)
            nc.gpsimd.collective_compute(
                "AllReduce", ALU.add, replica_groups=pair_groups,
                ins=[ykv_in[1].opt()], outs=[ykv_out[layer][1].opt()])

            # ---------------- phase 3: y_sparse/xy/decoder in 2 t-halves
            ym_acc0 = pat("pa", f"ym0_{layer}")
            tr1 = psA.tile([128, 1024], F16, name=f"tr1_{layer}", tag="pc")
            for nt in range(NT):
                phase3_nt(layer, nt, 0, ym_acc0)
                if nt == 20:
                    ykv_process(layer, 1, tr_tile=tr1)
            ym_store(0, ym_acc0)
            nc.gpsimd.collective_compute(
                "AllReduce", ALU.add, replica_groups=all_group,
                ins=[ym_in[0].opt()], outs=[ym_out[layer][0].opt()])

            ym_acc1 = pat("pb", f"ym1_{layer}")
            for nt in range(NT):
                phase3_nt(layer, nt, 1, ym_acc1)
                if nt == 20:
                    tail_chunk(layer, 0)
            ym_store(1, ym_acc1)
            nc.gpsimd.collective_compute(
                "AllReduce", ALU.add, replica_groups=all_group,
                ins=[ym_in[1].opt()], outs=[ym_out[layer][1].opt()])
            # tail_chunk(layer, 1) is issued at the top of the next layer's
            # pass A (overlaps wave A); for the last layer do it here.
            if layer == n_layer - 1:
                tail_chunk(layer, 1)

        # ------------------------------------------------------- lm head
        for tb in range(TB):
            th, tc_ = divmod(tb, 4)
            ps_l = psw(f"ps_lg_{tb}", (128, VOCAB))
            for d in range(DC):
                nc.tensor.matmul(
                    ps_l[:], x_d16[d][th][:, 128 * tc_:128 * (tc_ + 1)],
                    lmh_sb[d][:], start=(d == 0), stop=(d == DC - 1))
            lg_sb = wp.tile([128, VOCAB], F32, name="lg_sb", tag="lg_sb",
                            bufs=1)
            nc.vector.tensor_copy(lg_sb[:], ps_l[:])
            nc.sync.dma_start(out_o[128 * tb:128 * (tb + 1), :], lg_sb[:])

    nc.compile()
    return nc


# ------------------------------------------------------------- host helpers
def _host_tables():
    """cos/sin rope tables in [pair, t] layout, mirroring reference fp32 math."""
    n = np.arange(N, dtype=np.float32)
    q = np.floor(n / 2.0) * 2.0
    freqs = (1.0 / (np.float32(THETA) ** (q / np.float32(N)))
             / np.float32(2.0 * math.pi)).astype(np.float32)
    t = np.arange(T, dtype=np.float32)
    phases = (t[:, None] * freqs[None, :]) % 1.0
    phases = phases * np.float32(2.0 * math.pi)
    cos = np.cos(phases).astype(np.float32)   # [T, N]
    sin = np.sin(phases).astype(np.float32)
    # pair p uses freq of n=2p; table[p, t]
    cos_p = cos[:, 0::2].T.copy()  # [N//2, T]
    sin_p = sin[:, 0::2].T.copy()
    return cos_p, sin_p


def _perm_local():
    """Local latent permutation: position -> (pair index, odd flag)."""
    pos_to_pair = np.empty(NHALF, dtype=np.int64)
    pos_is_odd = np.empty(NHALF, dtype=np.int64)
    for j in range(NJ):
        pr = np.arange(128) + 128 * j
        pos_to_pair[256 * j:256 * j + 128] = pr
        pos_is_odd[256 * j:256 * j + 128] = 0
        pos_to_pair[256 * j + 128:256 * j + 256] = pr
        pos_is_odd[256 * j + 128:256 * j + 256] = 1
    return pos_to_pair, pos_is_odd


_NC_CACHE = {}


def _get_nc():
    if "nc" not in _NC_CACHE:
        _NC_CACHE["nc"] = build_program()
    return _NC_CACHE["nc"]


def prepare_in_maps(idx, embed, encoder, encoder_v, decoder, lm_head):
    idx = np.asarray(idx)
    embed = np.asarray(embed, dtype=np.float32)
    encoder = np.asarray(encoder, dtype=np.float32)
    encoder_v = np.asarray(encoder_v, dtype=np.float32)
    decoder = np.asarray(decoder, dtype=np.float32)
    lm_head = np.asarray(lm_head, dtype=np.float32)

    cos_p, sin_p = _host_tables()
    pos_to_pair, pos_is_odd = _perm_local()

    cmask = (np.arange(128)[:, None] < np.arange(128)[None, :]).astype(np.float16)
    ident = np.eye(128, dtype=np.float16)
    idx32 = idx.reshape(T).astype(np.float32).reshape(T, 1)
    lmh16 = lm_head.astype(np.float16)

    in_maps = []
    for c in range(NCORES):
        h, eta = c // 2, c % 2
        pair_g = NPAIR * eta + pos_to_pair          # global pair index
        n_orig = 2 * pair_g + pos_is_odd            # original n within head
        enc_sh = encoder[h][:, n_orig].astype(np.float16)
        encv_sh = encoder_v[h][:, n_orig].astype(np.float16)
        dec_sh = decoder[h * N + n_orig, :].astype(np.float16)
        cos_sh = cos_p[NPAIR * eta:NPAIR * (eta + 1), :].astype(np.float16)
        sin_sh = sin_p[NPAIR * eta:NPAIR * (eta + 1), :].astype(np.float16)
        in_maps.append({
            "idx32": idx32, "embed": embed, "enc_sh": enc_sh,
            "encv_sh": encv_sh, "dec_sh": dec_sh, "lmh": lmh16,
            "cos_sh": cos_sh, "sin_sh": sin_sh, "cmask": cmask,
            "ident": ident,
        })
    return in_maps


def kernel(idx, embed, encoder, encoder_v, decoder, lm_head):
    in_maps = prepare_in_maps(idx, embed, encoder, encoder_v, decoder,
                              lm_head)
    nc = _get_nc()
    res = bass_utils.run_bass_kernel_spmd(nc, in_maps,
                                          core_ids=list(range(NCORES)))
    _NC_CACHE["last_results"] = res
    logits = np.asarray(res.results[0]["logits"], dtype=np.float32)
    return logits.reshape(1, T, VOCAB)


# revision 20
# speedup vs baseline: 1.3067x; 1.0333x over previous
"""Trainium2 Bass kernel for nn_BDH_6313601925221 (sparse_attention).

Model (reference.py):
  x = LN(embed[idx])                                   (B=1, T=1024, D=256)
  repeat 6 layers (shared weights):
    x_sparse = relu(einsum('btd,hdn->bhtn', x, encoder))   N=8192, NH=4
    QR       = rope(x_sparse)                              interleaved-pair rotation
    scores   = einsum('bhtn,bhsn->bhts', QR, QR) * strict_causal
    yKV      = LN(einsum('bhts,bsd->bhtd', scores, x))
    y_sparse = relu(einsum('bhtd,hdn->bhtn', yKV, encoder_v))
    yMLP     = (x_sparse*y_sparse).transpose -> (T, NH*N) @ decoder
    x        = LN(x + LN(yMLP))
  logits = x @ lm_head

Distribution (8 cores): core c = (head h=c//2, latent-half eta=c%2).
Each core computes the encoder/rope/scores path over its 4096 latent dims.
Partial scores stay on-core (PSUM -> SBUF fp16); the contraction over the
latent halves is done by AllReducing the PARTIAL yKV within the head pair
(2 chunks of 4 token-blocks, pipelined against the remaining score strips).
y_sparse/xy/decoder run over the local latent half for all tokens; the
yMLP^T partials are AllReduced over all 8 ranks in 2 token-half chunks so
each AR hides under the other half's matmuls.

x is kept d-major ([128, 512] tiles per (dh, th)); the inter-layer LNs
compute stats along the partition (d) axis via ones-matmuls + PE
row-broadcasts, so the next layer's encoder matmuls start straight off the
AR result with no transpose chain. Token-major x (yKV rhs) is produced
lazily by 16 PE transposes.

QR storage (SBUF pressure): persistent QR_t (t-cols 512:1024) + QR_k3
(cols 384:512); the head (cols 0:384) is written IN PLACE into the xs
th0-half tiles (xs is spilled to DRAM before rope overwrites it). Score
strips therefore accumulate from up to 4 source chunks; within a PSUM
bank only the first chunk's nt==0 matmul uses start=True (clearing the
bank's has_written bits), later chunks' first matmuls use start=False and
overwrite-by-virgin-bit.

Layouts: latent dim N is host-permuted so rope pairs are de-interleaved:
local tile 2j = even pair members, 2j+1 = odd. Inner products over N and
the decoder contraction are invariant to this permutation (weights are
permuted to match).

PSUM (8 banks): psA tags pa/pb/pc (2 banks each) carry score-strip and
yKV/yMLP accumulations; psW (2 banks, bufs=2) carries transient matmul
outputs. The tail LN stats/broadcasts use pa generations so they never
block the ps_w FIFO that phase 3 depends on.
"""

import math
import sys

import numpy as np

for _p in ("/opt/trn_rl_repo",):
    if _p not in sys.path:
        sys.path.insert(0, _p)

import concourse.bass as bass
import concourse.mybir as mybir
import concourse.tile as tile
from concourse import bacc
from concourse import bass_utils

# ---------------------------------------------------------------- constants
D = 256
NH = 4
N = 8192
T = 1024
N_LAYER = 6
VOCAB = 256
THETA = 2 ** 16
EPS = 1e-5
NCORES = 8

NHALF = N // 2          # 4096 latent dims per core
NPAIR = NHALF // 2      # 2048 rope pairs per core
NT = NHALF // 128       # 32 local n-tiles of 128
NJ = NT // 2            # 16 pair-blocks (tile 2j = evens, 2j+1 = odds)
TB = T // 128           # 8 token blocks
DC = D // 128           # 2 d-chunks

F16 = mybir.dt.float16
F32 = mybir.dt.float32
I32 = mybir.dt.int32
AX = mybir.AxisListType
ALU = mybir.AluOpType
ACTF = mybir.ActivationFunctionType

INV_D = 1.0 / D


def _ln_free(nc, pool, x_ap, eps_ap, out_f16=None, n=None, name=""):
    """LayerNorm along the free dim of a [128, n] tile (per-partition stats)."""
    n = n if n is not None else x_ap.shape[-1]
    inv_n = 1.0 / n
    sq = pool.tile([128, n], F32, name=f"lnsq{name}", tag="lnsq")
    ssq = pool.tile([128, 1], F32, name=f"lnssq{name}", tag="lnssq")
    std = pool.tile([128, 1], F32, name=f"lnstd{name}", tag="lnstd")
    inv = pool.tile([128, 1], F32, name=f"lninv{name}", tag="lninv")
    mu = pool.tile([128, 1], F32, name=f"lnmu{name}", tag="lnmu")
    xm_t = pool.tile([128, n], F32, name=f"lnxm{name}", tag="lnxm")
    nc.vector.tensor_reduce(mu[:], x_ap, axis=AX.X, op=ALU.add)
    nc.scalar.mul(mu[:], mu[:], inv_n)
    nc.vector.tensor_scalar_sub(xm_t[:], x_ap, mu[:])
    xm = xm_t[:]
    nc.scalar.activation(sq[:], xm, ACTF.Square, accum_out=ssq[:])
    nc.scalar.activation(std[:], ssq[:], ACTF.Sqrt, bias=eps_ap, scale=inv_n)
    nc.vector.reciprocal(inv[:], std[:])
    if out_f16 is not None:
        nc.scalar.activation(out_f16, xm, ACTF.Copy, scale=inv[:])
    return xm, inv


def build_program(n_layer=N_LAYER):
    nc = bacc.Bacc("TRN2", target_bir_lowering=False, debug=False,
                   num_devices=NCORES)

    # ------------------------------------------------------------- I/O decl
    idx_i = nc.dram_tensor("idx32", [T, 1], F32, kind="ExternalInput")
    embed_i = nc.dram_tensor("embed", [VOCAB, D], F32, kind="ExternalInput")
    enc_i = nc.dram_tensor("enc_sh", [D, NHALF], F16, kind="ExternalInput")
    encv_i = nc.dram_tensor("encv_sh", [D, NHALF], F16, kind="ExternalInput")
    dec_i = nc.dram_tensor("dec_sh", [NHALF, D], F16, kind="ExternalInput")
    lmh_i = nc.dram_tensor("lmh", [D, VOCAB], F16, kind="ExternalInput")
    cos_i = nc.dram_tensor("cos_sh", [NPAIR, T], F16, kind="ExternalInput")
    sin_i = nc.dram_tensor("sin_sh", [NPAIR, T], F16, kind="ExternalInput")
    cmask_i = nc.dram_tensor("cmask", [128, 128], F16, kind="ExternalInput")
    ident_i = nc.dram_tensor("ident", [128, 128], F16, kind="ExternalInput")
    out_o = nc.dram_tensor("logits", [T, VOCAB], F32, kind="ExternalOutput")

    pair_groups = [[2 * h, 2 * h + 1] for h in range(NH)]
    all_group = [list(range(NCORES))]

    with tile.TileContext(nc) as tc:
      with (
        tc.tile_pool(name="persist", bufs=1) as pp,
        tc.tile_pool(name="work", bufs=2) as wp,
        tc.tile_pool(name="psW", bufs=2, space="PSUM") as psW,
        tc.tile_pool(name="psA", bufs=1, space="PSUM") as psA,
        tc.tile_pool(name="dram", bufs=1, space="DRAM") as dp,
      ):
        # ------------------------------------------------- persistent SBUF
        enc_sb = [pp.tile([128, NHALF], F16, name=f"enc{d}", tag=f"enc{d}")
                  for d in range(DC)]
        encv_sb = [pp.tile([128, NHALF], F16, name=f"encv{d}", tag=f"encv{d}")
                   for d in range(DC)]
        # QR tail (t-cols 512:1024) and k3 block (cols 384:512)
        QR_t = [pp.tile([128, 512], F16, name=f"qrt{i}", tag=f"qrt{i}")
                for i in range(NT)]
        QR_k3 = [pp.tile([128, 128], F16, name=f"qrk{i}", tag=f"qrk{i}")
                 for i in range(NT)]
        # score strips S^T[kb]: [s=128, (TB-kb)*128] fp16, masked on diag blk
        STS = [pp.tile([128, (TB - kb) * 128], F16, name=f"sts{kb}",
                       tag=f"sts{kb}") for kb in range(TB)]
        # x in d-major: per (dh, th) [128, 512] fp16 (residual carried fp16)
        x_d16 = [[pp.tile([128, 512], F16, name=f"xd16_{dh}_{th}",
                          tag=f"xd16_{dh}_{th}") for th in range(2)]
                 for dh in range(DC)]
        x_t16 = [pp.tile([128, D], F16, name=f"xt16_{i}", tag=f"xt16_{i}")
                 for i in range(TB)]
        ykvT = [[pp.tile([128, 512], F16, name=f"ykvT{d}_{th}",
                         tag=f"ykvT{d}_{th}") for th in range(2)]
                for d in range(DC)]
        # xs th0 halves for nt 0..15; after rope their cols [0:384] hold the
        # QR head (stationaries + moving chunks for score strips kb 0..2)
        xs_h0 = [pp.tile([128, 512], F16, name=f"xsh0_{i}", tag=f"xsh0_{i}")
                 for i in range(16)]
        cmask = pp.tile([128, 128], F16, name="cmaskt", tag="cmaskt")
        eps_t = pp.tile([128, 1], F32, name="eps_t", tag="eps_t")
        # eps for the yKV LN: partials are scaled by 1/16384, so the eps
        # added to their variance must scale by 1/16384^2 to match reference
        eps_yk = pp.tile([128, 1], F32, name="eps_yk", tag="eps_yk")
        ident = pp.tile([128, 128], F16, name="identt", tag="identt")
        ones_c = pp.tile([128, 1], F16, name="ones_c", tag="ones_c")
        ones_r = pp.tile([1, 128], F16, name="ones_r", tag="ones_r")
        ones4 = pp.tile([4, 128], F16, name="ones4", tag="ones4")
        lmh_sb = [pp.tile([128, VOCAB], F16, name=f"lmh{d}", tag=f"lmh{d}")
                  for d in range(DC)]

        # ---------------------------------------------------- DRAM buffers
        xs_spill = dp.tile([NHALF, T], F16, name="xs_spill")
        ykv_in = [dp.tile([512, D], F16, name=f"ykv_in{c}") for c in range(2)]
        ykv_out = [[dp.tile([512, D], F16, name=f"ykv_out{l}_{c}")
                    for c in range(2)]
                   for l in range(n_layer)]
        ym_in = [dp.tile([D, 512], F16, name=f"ym_in{c}") for c in range(2)]
        ym_out = [[dp.tile([D, 512], F16, name=f"ym_out{l}_{c}",
                           addr_space="Shared") for c in range(2)]
                  for l in range(n_layer)]

        def psw(name, shape=(128, 512), dtype=F32):
            return psW.tile(list(shape), dtype, name=name, tag="ps_w",
                            padded_shape=[128, 512])

        def pat(tag, name, dtype=F32):
            return psA.tile([128, 1024], dtype, name=name, tag=tag)

        # ------------------------------------------------------ load consts
        nc.gpsimd.memset(eps_t[:], EPS)
        nc.gpsimd.memset(eps_yk[:], EPS / (16384.0 * 16384.0))
        nc.gpsimd.memset(ones_c[:], 1.0)
        nc.gpsimd.memset(ones_r[:], 1.0)
        nc.gpsimd.memset(ones4[:], 1.0)
        nc.sync.dma_start(cmask[:], cmask_i[:, :])
        nc.sync.dma_start(ident[:], ident_i[:, :])
        for d in range(DC):
            nc.sync.dma_start(enc_sb[d][:], enc_i[128 * d:128 * (d + 1), :])
            nc.sync.dma_start(encv_sb[d][:], encv_i[128 * d:128 * (d + 1), :])
            nc.sync.dma_start(lmh_sb[d][:], lmh_i[128 * d:128 * (d + 1), :])

        # warm-up collectives: absorb the first-call CC overhead during
        # the embedding phase instead of layer 0's critical path
        cc_warm_in = dp.tile([128, 128], F16, name="ccw_in")
        cc_warm_o1 = dp.tile([128, 128], F16, name="ccw_o1")
        cc_warm_o2 = dp.tile([128, 128], F16, name="ccw_o2",
                             addr_space="Shared")
        warm_t = wp.tile([128, 128], F16, name="ccwt", tag="ccwt", bufs=1)
        nc.gpsimd.memset(warm_t[:], 0.0)
        nc.sync.dma_start(cc_warm_in[:, :], warm_t[:])
        nc.gpsimd.collective_compute(
            "AllReduce", ALU.add, replica_groups=pair_groups,
            ins=[cc_warm_in.opt()], outs=[cc_warm_o1.opt()])
        nc.gpsimd.collective_compute(
            "AllReduce", ALU.add, replica_groups=all_group,
            ins=[cc_warm_in.opt()], outs=[cc_warm_o2.opt()])

        # ------------------------------------------------------- embedding
        # E_n = LN(embed) per vocab row; x0 = onehot(idx) @ E_n
        with tc.tile_pool(name="embed", bufs=1) as ep:
            E_n = [ep.tile([128, D], F16, name=f"en{v}", tag=f"en{v}")
                   for v in range(DC)]
            for v in range(DC):
                emb_raw = ep.tile([128, D], F32, name=f"emb_raw{v}",
                                  tag="emb_raw")
                nc.sync.dma_start(emb_raw[:], embed_i[128 * v:128 * (v + 1), :])
                _ln_free(nc, ep, emb_raw[:], eps_t[:], out_f16=E_n[v][:],
                         name=f"emb{v}")

            iota_i32 = ep.tile([128, VOCAB], I32, name="iota_i32",
                               tag="iota_i32")
            nc.gpsimd.iota(iota_i32[:], pattern=[[1, VOCAB]], base=0,
                           channel_multiplier=0)
            iota_t = ep.tile([128, VOCAB], F32, name="iota_t", tag="iota_t")
            nc.vector.tensor_copy(iota_t[:], iota_i32[:])
            OHT = [ep.tile([128, T], F16, name=f"oht{v}", tag=f"oht{v}")
                   for v in range(DC)]
            for tb in range(TB):
                idx_col = ep.tile([128, 1], F32, name="idx_col",
                                  tag="idx_col", bufs=2)
                nc.sync.dma_start(idx_col[:], idx_i[128 * tb:128 * (tb + 1), :])
                oh_tm = ep.tile([128, VOCAB], F16, name="oh_tm", tag="oh_tm",
                                bufs=2)
                nc.vector.tensor_scalar(oh_tm[:], iota_t[:], idx_col[:], None,
                                        op0=ALU.is_equal)
                for v in range(DC):
                    ps_t = psw(f"ps_tr_oh{tb}_{v}", (128, 128), F16)
                    nc.tensor.transpose(ps_t[:],
                                        oh_tm[:, 128 * v:128 * (v + 1)],
                                        ident[:])
                    nc.scalar.copy(OHT[v][:, 128 * tb:128 * (tb + 1)], ps_t[:])

            # token-major x0 (for yKV rhs)
            for tb in range(TB):
                ps_x = psw(f"ps_x0_{tb}", (128, D))
                for v in range(DC):
                    nc.tensor.matmul(ps_x[:],
                                     OHT[v][:, 128 * tb:128 * (tb + 1)],
                                     E_n[v][:], start=(v == 0),
                                     stop=(v == DC - 1))
                nc.scalar.copy(x_t16[tb][:], ps_x[:])
            # d-major x0
            for d in range(DC):
                for th in range(2):
                    ps_xd = psw(f"ps_xd_{d}_{th}")
                    for v in range(DC):
                        nc.tensor.matmul(
                            ps_xd[:], E_n[v][:, 128 * d:128 * (d + 1)],
                            OHT[v][:, 512 * th:512 * (th + 1)],
                            start=(v == 0), stop=(v == DC - 1))
                    nc.scalar.copy(x_d16[d][th][:], ps_xd[:])

        # ------------------------------------------------------ helpers
        def enc_mm(nt, th, xs_dst_ap):
            """xs[nt][:, th] = relu(enc^T x) for one (nt, th)."""
            ps_e = psw(f"ps_enc_{nt}_{th}")
            for d in range(DC):
                nc.tensor.matmul(
                    ps_e[:], enc_sb[d][:, 128 * nt:128 * (nt + 1)],
                    x_d16[d][th][:], start=(d == 0), stop=(d == DC - 1))
            nc.scalar.activation(xs_dst_ap, ps_e[:], ACTF.Relu)

        def rope_half(j, th, xe_t, xo_t, ct_ap, st_ap, g_ops):
            """Rope for pair-block j, token half th.

            th==1: writes QR_t[2j]/QR_t[2j+1] (full [128,512] ops).
            th==0: writes the head back IN PLACE into xe_t/xo_t cols [0:384]
                   and the k3 block into QR_k3 (reads complete first).
            xe_t/xo_t are [128,512] tiles (th-half of xs for nt=2j, 2j+1).
            g_ops: subset of {'p2','p4'} to run on gpsimd."""
            p1 = wp.tile([128, 512], F16, name="rp1", tag="rp1", bufs=1)
            p2 = wp.tile([128, 512], F16, name="rp2", tag="rp2", bufs=2)
            p3 = wp.tile([128, 512], F16, name="rp3", tag="rp3", bufs=1)
            p4 = wp.tile([128, 512], F16, name="rp4", tag="rp4", bufs=2)
            nc.vector.tensor_mul(p1[:], xe_t[:], ct_ap)
            nc.vector.tensor_mul(p2[:], xo_t[:], st_ap)
            nc.vector.tensor_mul(p3[:], xo_t[:], ct_ap)
            nc.vector.tensor_mul(p4[:], xe_t[:], st_ap)
            if th == 1:
                nc.vector.tensor_sub(QR_t[2 * j][:], p1[:], p2[:])
                nc.vector.tensor_add(QR_t[2 * j + 1][:], p3[:], p4[:])
            else:
                nc.vector.tensor_sub(xe_t[:, 0:384], p1[:, 0:384],
                                     p2[:, 0:384])
                nc.vector.tensor_sub(QR_k3[2 * j][:], p1[:, 384:512],
                                     p2[:, 384:512])
                nc.vector.tensor_add(xo_t[:, 0:384], p3[:, 0:384],
                                     p4[:, 0:384])
                nc.vector.tensor_add(QR_k3[2 * j + 1][:], p3[:, 384:512],
                                     p4[:, 384:512])

        def load_costab(j, th):
            ct = wp.tile([128, 512], F16, name=f"cos{th}", tag=f"cos{th}",
                         bufs=3)
            st = wp.tile([128, 512], F16, name=f"sin{th}", tag=f"sin{th}",
                         bufs=3)
            cols = slice(512 * th, 512 * (th + 1))
            nc.sync.dma_start(ct[:], cos_i[128 * j:128 * (j + 1), cols])
            nc.sync.dma_start(st[:], sin_i[128 * j:128 * (j + 1), cols])
            return ct, st

        def strip_chunks(kb, head_ap, nt):
            """(src_ap, strip col offset, width) chunks for strip kb of nt.

            Strip kb covers t-cols [128*kb, 1024) => widths (TB-kb)*128.
            Sources: head (xs tile cols [128kb:384]), QR_k3, QR_t halves
            split at the psum bank boundary (strip col 512)."""
            out = []
            off = 0
            if kb <= 2:
                w = 384 - 128 * kb
                out.append((head_ap[:, 128 * kb:384], off, w))
                off += w
            if kb <= 3:
                out.append((QR_k3[nt][:], off, 128))
                off += 128
                # QR_t part(s), split at strip col 512
                b = 512 - off
                if b > 0:
                    out.append((QR_t[nt][:, 0:b], off, b))
                out.append((QR_t[nt][:, b:512], 512, 512 - b))
            else:
                c0 = 128 * (kb - 4)
                out.append((QR_t[nt][:, c0:512], 0, 512 - c0))
            return out

        def strip_station(kb, head_ap, nt):
            if kb <= 2:
                return head_ap[:, 128 * kb:128 * (kb + 1)]
            if kb == 3:
                return QR_k3[nt][:]
            return QR_t[nt][:, 128 * (kb - 4):128 * (kb - 3)]

        def strip_nt_mms(kb, dst_tile, head_ap, nt):
            """All matmuls of strip kb for one nt.

            A strip's chunks can share a psum bank, so per bank: start=True
            only on the first chunk at nt==0 (clears stale has_written; later
            chunks' first matmuls overwrite via their virgin bits), and
            stop=True only on the bank's last chunk at nt==NT-1 (stop clears
            the sim's group-started flag)."""
            stat = strip_station(kb, head_ap, nt)
            chunks = strip_chunks(kb, head_ap, nt)
            first_in_bank = {}
            last_in_bank = {}
            for i, (_, off, _) in enumerate(chunks):
                bank = off // 512
                first_in_bank.setdefault(bank, i)
                last_in_bank[bank] = i
            for i, (src_ap, off, w) in enumerate(chunks):
                bank = off // 512
                nc.tensor.matmul(
                    dst_tile[:, off:off + w], stat, src_ap,
                    start=(nt == 0 and first_in_bank[bank] == i),
                    stop=(nt == NT - 1 and last_in_bank[bank] == i))

        def strip_copy(kb, src_tile):
            """PSUM score strip kb -> STS[kb] fp16, diag block masked."""
            w = (TB - kb) * 128
            nc.vector.tensor_mul(STS[kb][:, 0:128], src_tile[:, 0:128],
                                 cmask[:])
            if w > 128:
                nc.scalar.copy(STS[kb][:, 128:w], src_tile[:, 128:w])

        def ykv_mms(qb, region_ap):
            """Partial yKV for token block qb into a psum region [128, 256]."""
            for kb in range(qb + 1):
                nc.tensor.matmul(
                    region_ap,
                    STS[kb][:, 128 * (qb - kb):128 * (qb - kb) + 128],
                    x_t16[kb][:], start=(kb == 0), stop=(kb == qb))

        def ykv_store(qb, region_ap, chunk):
            # yKV partials are pre-LN and can exceed fp16 range; LN is
            # scale-invariant, so store them scaled down by 1/16384
            # (keeps even the largest late-token rows' fp16 square terms
            # well inside range; row stds stay in normal fp16 range).
            stg = wp.tile([128, D], F16, name="ykvst", tag="ykvst")
            nc.scalar.activation(stg[:], region_ap, ACTF.Copy,
                                 scale=1.0 / 16384.0)
            q = qb % 4
            nc.sync.dma_start(ykv_in[chunk][128 * q:128 * (q + 1), :], stg[:])

        def ykv_process(layer, chunk, tr_tile=None):
            """Load AR'd yKV chunk, LN, transpose into ykvT[:][chunk].

            tr_tile: optional F16 psum tile for the transposes (so they do
            not share the ps_w FIFO with concurrently-running phase-3 tiles).
            """
            ld4 = wp.tile([128, 4, D], F16, name="ykvld4", tag="ykvld4",
                          bufs=1)
            for q in range(4):
                nc.sync.dma_start(
                    ld4[:, q, :],
                    ykv_out[layer][chunk][128 * q:128 * (q + 1), :])
            mu4 = wp.tile([128, 4], F32, name="ykmu", tag="ykmu", bufs=1)
            nc.vector.tensor_reduce(mu4[:], ld4[:], axis=AX.X, op=ALU.add)
            xm4 = wp.tile([128, 4, D], F16, name="ykxm", tag="ykxm", bufs=1)
            nc.vector.scalar_tensor_tensor(
                xm4[:], mu4[:, :, None].broadcast_to([128, 4, D]), -INV_D,
                ld4[:], op0=ALU.mult, op1=ALU.add)
            nc.vector.tensor_mul(ld4[:], xm4[:], xm4[:])
            ssq4 = wp.tile([128, 4], F32, name="ykssq", tag="ykssq", bufs=1)
            nc.vector.tensor_reduce(ssq4[:], ld4[:], axis=AX.X, op=ALU.add)
            std4 = wp.tile([128, 4], F32, name="ykstd", tag="ykstd", bufs=1)
            nc.scalar.activation(std4[:], ssq4[:], ACTF.Sqrt,
                                 bias=eps_yk[:], scale=INV_D)
            inv4 = wp.tile([128, 4], F32, name="ykinv", tag="ykinv", bufs=1)
            nc.vector.reciprocal(inv4[:], std4[:])
            nc.vector.tensor_mul(
                ld4[:], xm4[:], inv4[:, :, None].broadcast_to([128, 4, D]))
            yt4 = ld4
            for q in range(4):
                qb = 4 * chunk + q
                for d in range(DC):
                    if tr_tile is None:
                        ps_t = psw(f"ps_tr_ykv{qb}_{d}", (128, 128), F16)[:]
                    else:
                        c0 = 128 * (2 * q + d)
                        ps_t = tr_tile[:, c0:c0 + 128]
                    nc.tensor.transpose(
                        ps_t, yt4[:, q, 128 * d:128 * (d + 1)], ident[:])
                    nc.scalar.copy(ykvT[d][chunk][:, 128 * q:128 * (q + 1)],
                                   ps_t)

        def xt_transpose(tb):
            """x_t16[tb] from d-major x (2 PE transposes + copies)."""
            th, tc_ = divmod(tb, 4)
            for d in range(DC):
                ps_t = psw(f"ps_tr_x{tb}_{d}", (128, 128), F16)
                nc.tensor.transpose(
                    ps_t[:], x_d16[d][th][:, 128 * tc_:128 * (tc_ + 1)],
                    ident[:])
                nc.scalar.copy(x_t16[tb][:, 128 * d:128 * (d + 1)], ps_t[:])

        def phase3_nt(layer, nt, th, ym_acc):
            """y_sparse, xy, decoder partials for one (nt, th)."""
            dec_t = wp.tile([128, D], F16, name="dec_t", tag="dec_t", bufs=3)
            nc.sync.dma_start(dec_t[:], dec_i[128 * nt:128 * (nt + 1), :])
            xs_t = wp.tile([128, 512], F16, name="xs3", tag="xs3", bufs=3)
            nc.sync.dma_start(
                xs_t[:],
                xs_spill[128 * nt:128 * (nt + 1), 512 * th:512 * (th + 1)])
            ps_v = psw(f"ps_ysp_{layer}_{nt}_{th}")
            for d in range(DC):
                nc.tensor.matmul(
                    ps_v[:], encv_sb[d][:, 128 * nt:128 * (nt + 1)],
                    ykvT[d][th][:], start=(d == 0), stop=(d == DC - 1))
            ys = wp.tile([128, 512], F16, name="ys", tag="ys", bufs=2)
            if nt % 2 == 0:
                nc.scalar.activation(ys[:], ps_v[:], ACTF.Relu)
            else:
                nc.vector.tensor_scalar(ys[:], ps_v[:], 0.0, None,
                                        op0=ALU.max)
            xy = wp.tile([128, 512], F16, name="xy", tag="xy", bufs=2)
            nc.vector.tensor_mul(xy[:], ys[:], xs_t[:])
            for dh in range(DC):
                nc.tensor.matmul(
                    ym_acc[:, 512 * dh:512 * dh + 512],
                    dec_t[:, 128 * dh:128 * (dh + 1)],
                    xy[:], start=(nt == 0), stop=(nt == NT - 1))

        def ym_store(th, ym_acc):
            for dh in range(DC):
                stg = wp.tile([128, 512], F16, name="ymst", tag="ymst")
                nc.scalar.copy(stg[:], ym_acc[:, 512 * dh:512 * dh + 512])
                nc.sync.dma_start(
                    ym_in[th][128 * dh:128 * (dh + 1), :], stg[:])

        def bcast_stats(um_list, name):
            """Per-column LN affine (a, b) from the partition-dim stats of
            the two [128,512] tiles in um_list; returns psum broadcasts
            (fp32 -- the fp32 apply chain keeps the x*a + b cancellation
            exact; a/b rows are fp16 which only costs ~5e-4 relative).

            Uses pa psum generations (not ps_w) so the stats chain never
            blocks the ps_w FIFO that concurrently running phases depend on.
            """
            rows = pat("pa", f"r_{name}")
            mu_ps = rows[0:1, 0:512]
            s2_ps = rows[0:1, 512:1024]
            sq = []
            for dh in range(DC):
                nc.tensor.matmul(mu_ps, ones_c[:], um_list[dh],
                                 start=(dh == 0), stop=(dh == DC - 1))
                sqt = wp.tile([128, 512], F16, name="tsq", tag="tsq")
                nc.vector.tensor_mul(sqt[:], um_list[dh], um_list[dh])
                sq.append(sqt)
            for dh in range(DC):
                nc.tensor.matmul(s2_ps, ones_c[:], sq[dh][:],
                                 start=(dh == 0), stop=(dh == DC - 1))
            m_row = wp.tile([1, 512], F32, name="mrow", tag="mrow")
            nc.vector.tensor_scalar(m_row[:], mu_ps, INV_D, None,
                                    op0=ALU.mult)
            mm2 = wp.tile([1, 512], F32, name="mm2", tag="mm2")
            nc.vector.tensor_mul(mm2[:], m_row[:], m_row[:])
            var_r = wp.tile([1, 512], F32, name="varr", tag="varr")
            nc.vector.scalar_tensor_tensor(var_r[:], s2_ps, INV_D,
                                           mm2[:], op0=ALU.mult,
                                           op1=ALU.subtract)
            std_r = wp.tile([1, 512], F32, name="stdr", tag="stdr")
            nc.scalar.activation(std_r[:], var_r[:], ACTF.Sqrt,
                                 bias=eps_t[0:1, :])
            inv_r = wp.tile([1, 512], F32, name="invr", tag="invr")
            nc.vector.reciprocal(inv_r[:], std_r[:])
            a_row = wp.tile([1, 512], F16, name="arow", tag="arow")
            nc.vector.tensor_copy(a_row[:], inv_r[:])
            b_row = wp.tile([1, 512], F16, name="brow", tag="brow")
            nc.vector.scalar_tensor_tensor(b_row[:], m_row[:], -1.0, inv_r[:],
                                           op0=ALU.mult, op1=ALU.mult)
            bc = pat("pa", f"b_{name}")
            nc.tensor.matmul(bc[:, 0:512], ones_r[:], a_row[:], start=True,
                             stop=True)
            nc.tensor.matmul(bc[:, 512:1024], ones_r[:], b_row[:], start=True,
                             stop=True)
            return bc[:, 0:512], bc[:, 512:1024]

        def tail_chunk(layer, th):
            """x[:, th] = LN(x + LN(yMLP[:, th])) in d-major (fp16 residual)."""
            um = []
            for dh in range(DC):
                u = wp.tile([128, 512], F16, name=f"um{dh}", tag=f"um{dh}",
                            bufs=1)
                nc.sync.dma_start(
                    u[:], ym_out[layer][th][128 * dh:128 * (dh + 1), :])
                um.append(u)
            a_b, b_b = bcast_stats([u[:] for u in um], f"t1_{layer}_{th}")
            v16 = []
            for dh in range(DC):
                t1 = wp.tile([128, 512], F32, name="tz1", tag="tz1", bufs=1)
                nc.vector.tensor_mul(t1[:], um[dh][:], a_b)
                t2 = wp.tile([128, 512], F32, name="tz2", tag="tz2", bufs=1)
                nc.vector.tensor_add(t2[:], t1[:], b_b)
                v = wp.tile([128, 512], F16, name="tv16", tag=f"tv16{dh}",
                            bufs=1)
                nc.vector.tensor_add(v[:], t2[:], x_d16[dh][th][:])
                v16.append(v)
            a2_b, b2_b = bcast_stats([v[:] for v in v16], f"t2_{layer}_{th}")
            for dh in range(DC):
                u1 = wp.tile([128, 512], F32, name="tu1", tag="tu1", bufs=1)
                nc.vector.tensor_mul(u1[:], v16[dh][:], a2_b)
                nc.vector.tensor_add(x_d16[dh][th][:], u1[:], b2_b)

        # ============================================================ layers
        for layer in range(n_layer):
            # ---------------- pass A: enc waves + rope + score group 0
            # wave A: th0 for nt 0..15 (x th0 ready from tail_chunk(...,0))
            if layer > 0:
                for tb in range(4):
                    xt_transpose(tb)
            for nt in range(16):
                enc_mm(nt, 0, xs_h0[nt][:])
                nc.sync.dma_start(
                    xs_spill[128 * nt:128 * (nt + 1), 0:512], xs_h0[nt][:])
                if nt % 2 == 1:
                    j = nt // 2
                    ct, st = load_costab(j, 0)
                    rope_half(j, 0, xs_h0[nt - 1], xs_h0[nt],
                              ct[:], st[:], g_ops={'p2', 'p4'})
            # tail th1 of previous layer (vector chain; waits ym AR chunk 1)
            if layer > 0:
                tail_chunk(layer - 1, 1)
                for tb in range(4, 8):
                    xt_transpose(tb)
            # wave B: th1 for nt 0..15; rope th1; scores group0 skew 2
            acc = {0: pat("pa", f"sc0_{layer}"),
                   1: pat("pb", f"sc1_{layer}"),
                   2: pat("pc", f"sc2_{layer}")}
            head_of = {}

            def scores_g0(nt):
                for kb in range(3):
                    strip_nt_mms(kb, acc[kb], head_of[nt][:], nt)

            for nt in range(16):
                head_of[nt] = xs_h0[nt]
                xs1 = wp.tile([128, 512], F16, name="xsh1", tag="xsh1",
                              bufs=4)
                enc_mm(nt, 1, xs1[:])
                nc.sync.dma_start(
                    xs_spill[128 * nt:128 * (nt + 1), 512:1024], xs1[:])
                if nt % 2 == 0:
                    xs1_prev = xs1
                else:
                    j = nt // 2
                    ct, st = load_costab(j, 1)
                    rope_half(j, 1, xs1_prev, xs1, ct[:], st[:],
                              g_ops={'p2'})
                    if j >= 2:
                        scores_g0(2 * (j - 2))
                        scores_g0(2 * (j - 2) + 1)
            # nt 16..31: both th, rope both halves, scores skew 2
            for nt in range(16, 32):
                x0t = wp.tile([128, 512], F16, name="xs30", tag="xs30",
                              bufs=5)
                head_of[nt] = x0t
                enc_mm(nt, 0, x0t[:])
                nc.sync.dma_start(
                    xs_spill[128 * nt:128 * (nt + 1), 0:512], x0t[:])
                x1t = wp.tile([128, 512], F16, name="xs31", tag="xs31",
                              bufs=4)
                enc_mm(nt, 1, x1t[:])
                nc.sync.dma_start(
                    xs_spill[128 * nt:128 * (nt + 1), 512:1024], x1t[:])
                if nt % 2 == 0:
                    xs_prev = (x0t, x1t)
                else:
                    j = nt // 2
                    ct, st = load_costab(j, 1)
                    rope_half(j, 1, xs_prev[1], x1t, ct[:], st[:],
                              g_ops={'p2'})
                    ct, st = load_costab(j, 0)
                    rope_half(j, 0, xs_prev[0], x0t, ct[:], st[:],
                              g_ops={'p2'})
                    scores_g0(2 * (j - 2))
                    scores_g0(2 * (j - 2) + 1)
            for nt in range(28, 32):
                scores_g0(nt)

            # ---------------- pass B: strips kb 3..7 + partial yKV + AR
            for kb in range(3):
                strip_copy(kb, acc[kb])
            # pc freed by strip_copy(2) -> yKV pair tiles cycle through pc
            pcT = pat("pc", f"ykv01_{layer}")
            ykv_mms(0, pcT[:, 0:256])
            ykv_mms(1, pcT[:, 512:768])
            ykv_store(0, pcT[:, 0:256], 0)
            ykv_store(1, pcT[:, 512:768], 0)

            strip_tag = {3: "pa", 4: "pb", 5: "pa", 6: "pb", 7: "pa"}

            def strip_mms(kb):
                stile = pat(strip_tag[kb], f"st_{layer}_{kb}")
                for nt in range(NT):
                    strip_nt_mms(kb, stile, head_of[nt][:], nt)
                strip_copy(kb, stile)

            strip_mms(3)
            pcT = pat("pc", f"ykv23_{layer}")
            ykv_mms(2, pcT[:, 0:256])
            ykv_mms(3, pcT[:, 512:768])
            ykv_store(2, pcT[:, 0:256], 0)
            ykv_store(3, pcT[:, 512:768], 0)
            nc.gpsimd.collective_compute(
                "AllReduce", ALU.add, replica_groups=pair_groups,
                ins=[ykv_in[0].opt()], outs=[ykv_out[layer][0].opt()])

            strip_mms(4)
            pcT = pat("pc", f"ykv45_{layer}")
            ykv_mms(4, pcT[:, 0:256])
            strip_mms(5)
            ykv_mms(5, pcT[:, 512:768])
            ykv_store(4, pcT[:, 0:256], 1)
            ykv_store(5, pcT[:, 512:768], 1)
            strip_mms(6)
            pcT = pat("pc", f"ykv67_{layer}")
            ykv_mms(6, pcT[:, 0:256])
            # yKV chunk 0 is back around here: LN + transpose into ykvT th0
            ykv_process(layer, 0)
            strip_mms(7)
            ykv_mms(7, pcT[:, 512:768])
            ykv_store(6, pcT[:, 0:256], 1)
            ykv_store(7, pcT[:, 512:768], 1)
            nc.gpsimd.collective_compute(
                "AllReduce", ALU.add, replica_groups=pair_groups,
                ins=[ykv_in[1].opt()], outs=[ykv_out[layer][1].opt()])

            # ---------------- phase 3: y_sparse/xy/decoder in 2 t-halves
            ym_acc0 = pat("pa", f"ym0_{layer}")
            tr1 = psA.tile([128, 1024], F16, name=f"tr1_{layer}", tag="pc")
            for nt in range(NT):
                phase3_nt(layer, nt, 0, ym_acc0)
                if nt == 20:
                    ykv_process(layer, 1, tr_tile=tr1)
            ym_store(0, ym_acc0)
            nc.gpsimd.collective_compute(
                "AllReduce", ALU.add, replica_groups=all_group,
                ins=[ym_in[0].opt()], outs=[ym_out[layer][0].opt()])

            ym_acc1 = pat("pb", f"ym1_{layer}")
            for nt in range(NT):
                phase3_nt(layer, nt, 1, ym_acc1)
                if nt == 20:
                    tail_chunk(layer, 0)
            ym_store(1, ym_acc1)
            nc.gpsimd.collective_compute(
                "AllReduce", ALU.add, replica_groups=all_group,
                ins=[ym_in[1].opt()], outs=[ym_out[layer][1].opt()])
            # tail_chunk(layer, 1) is issued at the top of the next layer's
            # pass A (overlaps wave A); for the last layer do it here.
            if layer == n_layer - 1:
                tail_chunk(layer, 1)

        # ------------------------------------------------------- lm head
        for tb in range(TB):
            th, tc_ = divmod(tb, 4)
            ps_l = psw(f"ps_lg_{tb}", (128, VOCAB))
            for d in range(DC):
                nc.tensor.matmul(
                    ps_l[:], x_d16[d][th][:, 128 * tc_:128 * (tc_ + 1)],
                    lmh_sb[d][:], start=(d == 0), stop=(d == DC - 1))
            lg_sb = wp.tile([128, VOCAB], F32, name="lg_sb", tag="lg_sb",
                            bufs=1)
            nc.vector.tensor_copy(lg_sb[:], ps_l[:])
            nc.sync.dma_start(out_o[128 * tb:128 * (tb + 1), :], lg_sb[:])

    nc.compile()
    return nc


# ------------------------------------------------------------- host helpers
def _host_tables():
    """cos/sin rope tables in [pair, t] layout, mirroring reference fp32 math."""
    n = np.arange(N, dtype=np.float32)
    q = np.floor(n / 2.0) * 2.0
    freqs = (1.0 / (np.float32(THETA) ** (q / np.float32(N)))
             / np.float32(2.0 * math.pi)).astype(np.float32)
    t = np.arange(T, dtype=np.float32)
    phases = (t[:, None] * freqs[None, :]) % 1.0
    phases = phases * np.float32(2.0 * math.pi)
    cos = np.cos(phases).astype(np.float32)   # [T, N]
    sin = np.sin(phases).astype(np.float32)
    # pair p uses freq of n=2p; table[p, t]
    cos_p = cos[:, 0::2].T.copy()  # [N//2, T]
    sin_p = sin[:, 0::2].T.copy()
    return cos_p, sin_p


def _perm_local():
    """Local latent permutation: position -> (pair index, odd flag)."""
    pos_to_pair = np.empty(NHALF, dtype=np.int64)
    pos_is_odd = np.empty(NHALF, dtype=np.int64)
    for j in range(NJ):
        pr = np.arange(128) + 128 * j
        pos_to_pair[256 * j:256 * j + 128] = pr
        pos_is_odd[256 * j:256 * j + 128] = 0
        pos_to_pair[256 * j + 128:256 * j + 256] = pr
        pos_is_odd[256 * j + 128:256 * j + 256] = 1
    return pos_to_pair, pos_is_odd


_NC_CACHE = {}


def _get_nc():
    if "nc" not in _NC_CACHE:
        _NC_CACHE["nc"] = build_program()
    return _NC_CACHE["nc"]


def prepare_in_maps(idx, embed, encoder, encoder_v, decoder, lm_head):
    idx = np.asarray(idx)
    embed = np.asarray(embed, dtype=np.float32)
    encoder = np.asarray(encoder, dtype=np.float32)
    encoder_v = np.asarray(encoder_v, dtype=np.float32)
    decoder = np.asarray(decoder, dtype=np.float32)
    lm_head = np.asarray(lm_head, dtype=np.float32)

    cos_p, sin_p = _host_tables()
    pos_to_pair, pos_is_odd = _perm_local()

    cmask = (np.arange(128)[:, None] < np.arange(128)[None, :]).astype(np.float16)
    ident = np.eye(128, dtype=np.float16)
    idx32 = idx.reshape(T).astype(np.float32).reshape(T, 1)
    lmh16 = lm_head.astype(np.float16)

    in_maps = []
    for c in range(NCORES):
        h, eta = c // 2, c % 2
        pair_g = NPAIR * eta + pos_to_pair          # global pair index
        n_orig = 2 * pair_g + pos_is_odd            # original n within head
        enc_sh = encoder[h][:, n_orig].astype(np.float16)
        encv_sh = encoder_v[h][:, n_orig].astype(np.float16)
        dec_sh = decoder[h * N + n_orig, :].astype(np.float16)
        cos_sh = cos_p[NPAIR * eta:NPAIR * (eta + 1), :].astype(np.float16)
        sin_sh = sin_p[NPAIR * eta:NPAIR * (eta + 1), :].astype(np.float16)
        in_maps.append({
            "idx32": idx32, "embed": embed, "enc_sh": enc_sh,
            "encv_sh": encv_sh, "dec_sh": dec_sh, "lmh": lmh16,
            "cos_sh": cos_sh, "sin_sh": sin_sh, "cmask": cmask,
            "ident": ident,
        })
    return in_maps


def kernel(idx, embed, encoder, encoder_v, decoder, lm_head):
    in_maps = prepare_in_maps(idx, embed, encoder, encoder_v, decoder,
                              lm_head)
    nc = _get_nc()
    res = bass_utils.run_bass_kernel_spmd(nc, in_maps,
                                          core_ids=list(range(NCORES)))
    _NC_CACHE["last_results"] = res
    logits = np.asarray(res.results[0]["logits"], dtype=np.float32)
    return logits.reshape(1, T, VOCAB)
